# revision 1
# baseline (speedup 1.0000x reference)
"""NonLocalBlock (GroupNorm + single-head 4096x4096 attention + residual)
Trainium2 Bass kernel, data-parallel over batch: 1 image per NeuronCore x8.

Per image (x: [512, 4096] channels-major):
  pass0: GroupNorm stats (bn_stats per channel, group-combine via tiny matmuls)
  passA: per hw-chunk of 512: normalize -> q/k (fp32 matmul, split to fp16
         hi/lo pairs), vT (fp32r). k hi/lo resident in SBUF, vT resident,
         q hi/lo spilled to DRAM.
  attention per group of 4 q-tiles (128 rows each):
         logits = 3-pass fp16-split matmul (hi*hi + hi*lo + lo*hi), softmax
         via ACT exp (accum_out row sums, fp32r probs), PE-transpose probs
         (fp32r), attn@v fp32r, transpose attn_h, batched output projection
         fp32r at N=512, +bias +residual, contiguous stores.
  bv is folded into attn_h (softmax weights sum to 1).
"""
import sys

sys.path.insert(0, '/opt/trn_rl_repo')
import numpy as np
import concourse.bass as bass
import concourse.bacc as bacc
import concourse.mybir as mybir
import concourse.tile as tile
from concourse.bass_utils import run_bass_kernel_spmd

F32 = mybir.dt.float32
F32R = mybir.dt.float32r
F16 = mybir.dt.float16
AF = mybir.ActivationFunctionType
AX = mybir.AxisListType
OP = mybir.AluOpType

C = 512
HW = 4096
NT = 4            # channel tiles of 128
NCH = 8           # hw chunks of 512
NQT = 32          # q tiles of 128
NG = 8            # groups of 4 q-tiles
GSIZE = 16        # channels per group
EPS = 1e-5
SCALE = float(np.float32(512.0) ** 0.5)


def build():
    nc = bacc.Bacc('TRN2', target_bir_lowering=False, debug=False)

    x_in = nc.declare_dram_parameter("x", [C, HW], F32, isOutput=False)
    wqh_in = nc.declare_dram_parameter("wqTh", [C, C], F16, isOutput=False)
    wql_in = nc.declare_dram_parameter("wqTl", [C, C], F16, isOutput=False)
    wkh_in = nc.declare_dram_parameter("wkTh", [C, C], F16, isOutput=False)
    wkl_in = nc.declare_dram_parameter("wkTl", [C, C], F16, isOutput=False)
    wvT_in = nc.declare_dram_parameter("wvT", [C, C], F32, isOutput=False)
    woT_in = nc.declare_dram_parameter("woT", [C, C], F32, isOutput=False)
    bias_in = nc.declare_dram_parameter("biases", [128, 16], F32,
                                        isOutput=False)  # bq|bk|bv|bo as [128,4]
    gb_in = nc.declare_dram_parameter("gammabeta", [128, 8], F32,
                                      isOutput=False)  # gamma|beta as [128,4]
    brow_in = nc.declare_dram_parameter("bias_rows", [2, C], F32,
                                        isOutput=False)  # bq|bk natural order
    out_dram = nc.declare_dram_parameter("out", [C, HW], F32, isOutput=True)

    qhi_dram = nc.dram_tensor("qhi_scratch", [C, HW], F16)
    qlo_dram = nc.dram_tensor("qlo_scratch", [C, HW], F16)

    a16 = np.zeros((128, 8), np.float32)
    for p in range(128):
        a16[p, p // GSIZE] = 1.0 / GSIZE
    b8 = np.zeros((8, 128), np.float32)
    for p in range(128):
        b8[p // GSIZE, p] = 1.0
    a16_d = nc.inline_tensor(a16, "a16")
    b8_d = nc.inline_tensor(b8, "b8")
    ident_d = nc.inline_tensor(np.eye(128, dtype=np.float32), "ident128")
    ones_d = nc.inline_tensor(np.ones((1, 512), np.float32), "ones512")

    with tile.TileContext(nc) as tc:
        with (
            tc.tile_pool(name="res", bufs=1) as res,
            tc.tile_pool(name="pp_proj", bufs=2, space="PSUM") as pp_proj,
            tc.tile_pool(name="pp_log", bufs=3, space="PSUM") as pp_log,
            tc.tile_pool(name="pp_tr", bufs=2, space="PSUM") as pp_tr,
            tc.tile_pool(name="pp_attn", bufs=1, space="PSUM") as pp_attn,
        ):
            # ---------- residents ----------
            khi_res = [res.tile([128, HW], F16, tag=f"khi{t}", name=f"khi{t}")
                       for t in range(NT)]
            klo_res = [res.tile([128, HW], F16, tag=f"klo{t}", name=f"klo{t}")
                       for t in range(NT)]
            vT_res = [res.tile([128, C], F32R, tag=f"vT{m}", name=f"vT{m}")
                      for m in range(NQT)]
            wo_sb = [res.tile([128, C], F32R, tag=f"wo{t}", name=f"wo{t}")
                     for t in range(NT)]
            for t in range(NT):
                nc.gpsimd.dma_start(out=wo_sb[t],
                                    in_=woT_in[128 * t:128 * (t + 1), :])
            biases = res.tile([128, 16], F32, tag="biases")
            nc.sync.dma_start(out=biases, in_=bias_in[:])
            bq = biases[:, 0:4]
            bk = biases[:, 4:8]
            bv = biases[:, 8:12]
            bo = biases[:, 12:16]
            gmbt = res.tile([128, 8], F32, tag="gmbt")
            nc.sync.dma_start(out=gmbt, in_=gb_in[:])
            gam = gmbt[:, 0:4]
            bet = gmbt[:, 4:8]
            a16_sb = res.tile([128, 8], F32, tag="a16")
            nc.sync.dma_start(out=a16_sb, in_=a16_d[:])
            b8_sb = res.tile([8, 128], F32, tag="b8")
            nc.sync.dma_start(out=b8_sb, in_=b8_d[:])
            id_sb = res.tile([128, 128], F32, tag="ident")
            nc.sync.dma_start(out=id_sb, in_=ident_d[:])
            idr_sb = res.tile([128, 128], F32R, tag="identr")
            nc.gpsimd.dma_start(out=idr_sb, in_=ident_d[:])
            bq_row = res.tile([1, C], F32R, tag="bq_row")
            nc.gpsimd.dma_start(out=bq_row, in_=brow_in[0:1, :])
            bk_row = res.tile([1, C], F32R, tag="bk_row")
            nc.gpsimd.dma_start(out=bk_row, in_=brow_in[1:2, :])
            ones_row = res.tile([1, 512], F32R, tag="ones_row")
            nc.gpsimd.dma_start(out=ones_row, in_=ones_d[:])
            eps8 = res.tile([8, 1], F32, tag="eps8")
            nc.vector.memset(eps8, EPS)
            scale_sb = res.tile([128, NT], F32, tag="scale")
            shift_sb = res.tile([128, NT], F32, tag="shift")

            # PE warmup: ~5us of dummy transposes while pass0 stats run,
            # so HAM unthrottles (1.2->2.4GHz) before passA matmuls start.
            wps = pp_log.tile([128, 128], F32, tag="ps_l", name="wps")
            for _ in range(24):
                nc.tensor.transpose(wps, id_sb, id_sb)

            # ---------- pass 0: GroupNorm statistics ----------
            with tc.tile_pool(name="p0", bufs=4) as p0, \
                 tc.tile_pool(name="p0s", bufs=1) as p0s:
                st6 = p0s.tile([128, NT, NCH, 6], F32, tag="st6")
                for n in range(NCH):
                    for t in range(NT):
                        xc = p0.tile([128, 512], F32, tag="x0")
                        nc.sync.dma_start(
                            out=xc,
                            in_=x_in[128 * t:128 * (t + 1), 512 * n:512 * (n + 1)])
                        nc.vector.bn_stats(out=st6[:, t, n, :], in_=xc)
                mv = p0s.tile([128, NT, 2], F32, tag="mv")
                for t in range(NT):
                    nc.vector.bn_aggr(out=mv[:, t, :], in_=st6[:, t, :, :])
                # stats_in: cols 0-3 mean_t, cols 4-7 E[x^2]_t
                stats_in = p0s.tile([128, 8], F32, tag="stats_in")
                for t in range(NT):
                    nc.vector.tensor_copy(stats_in[:, t:t + 1], mv[:, t, 0:1])
                    nc.vector.tensor_mul(stats_in[:, 4 + t:5 + t],
                                         mv[:, t, 0:1], mv[:, t, 0:1])
                    nc.vector.tensor_add(stats_in[:, 4 + t:5 + t],
                                         stats_in[:, 4 + t:5 + t], mv[:, t, 1:2])
                ps_g = pp_proj.tile([8, 8], F32, tag="ps_proj")
                nc.tensor.matmul(ps_g, a16_sb, stats_in, start=True, stop=True)
                g_sb = p0s.tile([8, 8], F32, tag="g_sb")
                nc.vector.tensor_copy(g_sb, ps_g)
                # group var = E[x^2]_g - mean_g^2 ; rstd = exp(-0.5*ln(var+eps))
                var_g = p0s.tile([8, 4], F32, tag="var_g")
                nc.vector.tensor_mul(var_g, g_sb[:, 0:4], g_sb[:, 0:4])
                nc.vector.tensor_tensor(out=var_g, in0=g_sb[:, 4:8], in1=var_g,
                                        op=OP.subtract)
                bc_in = p0s.tile([8, 8], F32, tag="bc_in")
                nc.vector.tensor_copy(bc_in[:, 0:4], g_sb[:, 0:4])
                nc.scalar.activation(out=bc_in[:, 4:8], in_=var_g, func=AF.Ln,
                                     bias=eps8, scale=1.0)
                nc.scalar.activation(out=bc_in[:, 4:8], in_=bc_in[:, 4:8],
                                     func=AF.Exp, bias=0.0, scale=-0.5)
                ps_bc = pp_proj.tile([128, 8], F32, tag="ps_proj")
                nc.tensor.matmul(ps_bc, b8_sb, bc_in, start=True, stop=True)
                chan = p0s.tile([128, 8], F32, tag="chan")
                nc.vector.tensor_copy(chan, ps_bc)
                # scale = gamma * rstd ; shift = beta - mean*scale
                nc.vector.tensor_mul(scale_sb, gam, chan[:, 4:8])
                tmp = p0s.tile([128, NT], F32, tag="tmp")
                nc.vector.tensor_mul(tmp, chan[:, 0:4], scale_sb)
                nc.vector.tensor_tensor(out=shift_sb, in0=bet, in1=tmp,
                                        op=OP.subtract)

            # ---------- pass A: hidden -> q(hi/lo), k(hi/lo), vT ----------
            with tc.tile_pool(name="pa_w", bufs=1) as pa_w, \
                 tc.tile_pool(name="pa_x", bufs=3) as pa_x, \
                 tc.tile_pool(name="pa_h", bufs=8) as pa_h, \
                 tc.tile_pool(name="pa_hr", bufs=4) as pa_hr, \
                 tc.tile_pool(name="pa_q", bufs=2) as pa_q:
                wqh_sb = [pa_w.tile([128, C], F16, tag=f"wqh{t}", name=f"wqh{t}")
                          for t in range(NT)]
                wql_sb = [pa_w.tile([128, C], F16, tag=f"wql{t}", name=f"wql{t}")
                          for t in range(NT)]
                wkh_sb = [pa_w.tile([128, C], F16, tag=f"wkh{t}", name=f"wkh{t}")
                          for t in range(NT)]
                wkl_sb = [pa_w.tile([128, C], F16, tag=f"wkl{t}", name=f"wkl{t}")
                          for t in range(NT)]
                wv_sb = [pa_w.tile([128, C], F32R, tag=f"wv{t}", name=f"wv{t}")
                         for t in range(NT)]
                for t in range(NT):
                    sl = slice(128 * t, 128 * (t + 1))
                    nc.sync.dma_start(out=wqh_sb[t], in_=wqh_in[sl, :])
                    nc.sync.dma_start(out=wql_sb[t], in_=wql_in[sl, :])
                    nc.sync.dma_start(out=wkh_sb[t], in_=wkh_in[sl, :])
                    nc.sync.dma_start(out=wkl_sb[t], in_=wkl_in[sl, :])
                    nc.gpsimd.dma_start(out=wv_sb[t], in_=wvT_in[sl, :])
                for n in range(NCH):
                    cols = slice(512 * n, 512 * (n + 1))
                    hid = []
                    hid_r = []
                    hhi = []
                    hlo = []
                    for t in range(NT):
                        xc = pa_x.tile([128, 512], F32, tag="xA")
                        nc.sync.dma_start(
                            out=xc, in_=x_in[128 * t:128 * (t + 1), cols])
                        h = pa_h.tile([128, 512], F32, tag="hid", bufs=4)
                        nc.vector.tensor_scalar(
                            out=h, in0=xc,
                            scalar1=scale_sb[:, t:t + 1],
                            scalar2=shift_sb[:, t:t + 1],
                            op0=OP.mult, op1=OP.add)
                        hid.append(h)
                        hr = pa_hr.tile([128, 512], F32R, tag="hid_r")
                        nc.gpsimd.tensor_copy(out=hr, in_=h)
                        hid_r.append(hr)
                        hh = pa_h.tile([128, 512], F16, tag="hhi", name="hh", bufs=5)
                        nc.scalar.copy(out=hh, in_=h)
                        hhi.append(hh)
                        hl = pa_h.tile([128, 512], F16, tag="hlo", name="hl", bufs=5)
                        nc.vector.tensor_tensor(out=hl, in0=h, in1=hh,
                                                op=OP.subtract)
                        hlo.append(hl)
                    # q and k projections (fp32), then split into fp16 hi/lo
                    # vT (fp32r): out[hw_t 128, c 512] = hidden_chunk_t.T @ wvT
                    # (bv folded into attn_h later: softmax weights sum to 1)
                    for t in range(NT):
                        ps = pp_proj.tile([128, 512], F32, tag="ps_proj")
                        for kc in range(NT):
                            nc.tensor.matmul(
                                ps, hid_r[kc][:, 128 * t:128 * (t + 1)],
                                wv_sb[kc], start=(kc == 0), stop=(kc == 3))
                        nc.vector.tensor_copy(vT_res[4 * n + t], ps)

                    for (w_h, w_l, b_row, is_q) in (
                            (wqh_sb, wql_sb, bq_row, True),
                            (wkh_sb, wkl_sb, bk_row, False)):
                        for m in range(NT):
                            ms = slice(128 * m, 128 * (m + 1))
                            ps = pp_proj.tile([128, 512], F32, tag="ps_proj")
                            for kc in range(NT):
                                nc.tensor.matmul(
                                    ps, w_h[kc][:, ms], hhi[kc],
                                    start=(kc == 0), stop=False)
                            for kc in range(NT):
                                nc.tensor.matmul(
                                    ps, w_h[kc][:, ms], hlo[kc],
                                    start=False, stop=False)
                            for kc in range(NT):
                                nc.tensor.matmul(
                                    ps, w_l[kc][:, ms], hhi[kc],
                                    start=False, stop=False)
                            nc.tensor.matmul(ps, b_row[:, ms], ones_row,
                                             start=False, stop=True)
                            if is_q:
                                hi = pa_q.tile([128, 512], F16, tag="qhi_st")
                                lo = pa_q.tile([128, 512], F16, tag="qlo_st")
                            else:
                                hi = khi_res[m][:, cols]
                                lo = klo_res[m][:, cols]
                            nc.scalar.copy(out=hi, in_=ps)
                            nc.vector.tensor_tensor(out=lo, in0=ps, in1=hi,
                                                    op=OP.subtract)
                            if is_q:
                                nc.sync.dma_start(
                                    out=qhi_dram[128 * m:128 * (m + 1), cols],
                                    in_=hi)
                                nc.sync.dma_start(
                                    out=qlo_dram[128 * m:128 * (m + 1), cols],
                                    in_=lo)
            # ---------- attention ----------
            with tc.tile_pool(name="at_q", bufs=2) as at_q, \
                 tc.tile_pool(name="at_l", bufs=1) as at_l, \
                 tc.tile_pool(name="at_p", bufs=3) as at_p, \
                 tc.tile_pool(name="at_pt", bufs=3) as at_pt, \
                 tc.tile_pool(name="at_s", bufs=2) as at_s, \
                 tc.tile_pool(name="at_h4", bufs=1) as at_h4, \
                 tc.tile_pool(name="at_o", bufs=2) as at_o:
                for g in range(NG):
                    attnh4 = at_h4.tile([128, NT, 512], F32R, tag="attnh4")
                    for qq in range(4):
                        qt = 4 * g + qq
                        qcols = slice(128 * qt, 128 * (qt + 1))
                        qtile_hi = at_q.tile([128, NT, 128], F16, tag="qtile_hi")
                        nc.sync.dma_start(
                            out=qtile_hi,
                            in_=qhi_dram[:, qcols].rearrange(
                                "(t p) q -> p t q", p=128))
                        qtile_lo = at_q.tile([128, NT, 128], F16, tag="qtile_lo")
                        nc.sync.dma_start(
                            out=qtile_lo,
                            in_=qlo_dram[:, qcols].rearrange(
                                "(t p) q -> p t q", p=128))

                        logits = at_l.tile([128, HW], F32, tag="logits")
                        maxs = at_s.tile([128, NCH], F32, tag="maxs")
                        for n in range(NCH):
                            ncols = slice(512 * n, 512 * (n + 1))
                            ps_l = pp_log.tile([128, 512], F32, tag="ps_l")
                            for kc in range(NT):
                                nc.tensor.matmul(
                                    ps_l, qtile_hi[:, kc, :],
                                    khi_res[kc][:, ncols],
                                    start=(kc == 0), stop=False)
                            for kc in range(NT):
                                nc.tensor.matmul(
                                    ps_l, qtile_hi[:, kc, :],
                                    klo_res[kc][:, ncols],
                                    start=False, stop=False)
                            for kc in range(NT):
                                nc.tensor.matmul(
                                    ps_l, qtile_lo[:, kc, :],
                                    khi_res[kc][:, ncols],
                                    start=False, stop=(kc == 3))
                            nc.vector.reduce_max(out=maxs[:, n:n + 1], in_=ps_l,
                                                 axis=AX.X)
                            nc.scalar.copy(out=logits[:, ncols], in_=ps_l)
                        negmax = at_s.tile([128, 1], F32, tag="negmax")
                        nc.vector.reduce_max(out=negmax, in_=maxs, axis=AX.X,
                                             negate=True)
                        negmax_s = at_s.tile([128, 1], F32, tag="negmax_s")
                        nc.vector.tensor_scalar_mul(out=negmax_s, in0=negmax,
                                                    scalar1=SCALE)
                        sums = at_s.tile([128, NCH], F32, tag="sums")
                        ps_at = pp_attn.tile([128, C], F32, tag="ps_at")
                        for n in range(NCH):
                            probs = at_p.tile([128, 512], F32R, tag="probs")
                            nc.scalar.activation(
                                out=probs, in_=logits[:, 512 * n:512 * (n + 1)],
                                func=AF.Exp, bias=negmax_s, scale=SCALE,
                                accum_out=sums[:, n:n + 1])
                            ps_t = pp_tr.tile([128, 512], F32R, tag="ps_t")
                            for j in range(4):
                                nc.tensor.transpose(
                                    ps_t[:, 128 * j:128 * (j + 1)],
                                    probs[:, 128 * j:128 * (j + 1)], idr_sb)
                            pT = at_pt.tile([128, 512], F32R, tag="pT")
                            nc.vector.tensor_copy(pT, ps_t)
                            for j in range(4):
                                nc.tensor.matmul(
                                    ps_at, pT[:, 128 * j:128 * (j + 1)],
                                    vT_res[4 * n + j],
                                    start=(n == 0 and j == 0),
                                    stop=(n == 7 and j == 3))
                        rowsum = at_s.tile([128, 1], F32, tag="rowsum")
                        nc.vector.reduce_sum(out=rowsum, in_=sums, axis=AX.X)
                        rinv = at_s.tile([128, 1], F32, tag="rinv")
                        nc.vector.reciprocal(out=rinv, in_=rowsum)
                        attn = at_s.tile([128, C], F32, tag="attn")
                        nc.vector.tensor_scalar_mul(out=attn, in0=ps_at,
                                                    scalar1=rinv)
                        # transpose attn [q, c] -> attn_h [c, q]; add bv
                        ps_t2 = pp_tr.tile([128, 512], F32, tag="ps_t")
                        for i in range(NT):
                            nc.tensor.transpose(
                                ps_t2[:, 128 * i:128 * (i + 1)],
                                attn[:, 128 * i:128 * (i + 1)], id_sb)
                        for i in range(NT):
                            nc.vector.tensor_scalar_add(
                                out=attnh4[:, i, 128 * qq:128 * (qq + 1)],
                                in0=ps_t2[:, 128 * i:128 * (i + 1)],
                                scalar1=bv[:, i:i + 1])
                    # batched output projection (fp32r, N=512) + bias + residual
                    gcols = slice(512 * g, 512 * (g + 1))
                    for m in range(NT):
                        ps_o = pp_proj.tile([128, 512], F32, tag="ps_proj")
                        for kc in range(NT):
                            nc.tensor.matmul(
                                ps_o, wo_sb[kc][:, 128 * m:128 * (m + 1)],
                                attnh4[:, kc, :], start=(kc == 0), stop=(kc == 3))
                        o_sb = at_o.tile([128, 512], F32, tag="o_sb")
                        nc.vector.tensor_scalar_add(
                            out=o_sb, in0=ps_o, scalar1=bo[:, m:m + 1])
                        xres = at_o.tile([128, 512], F32, tag="xres")
                        nc.sync.dma_start(
                            out=xres, in_=x_in[128 * m:128 * (m + 1), gcols])
                        nc.vector.tensor_add(out=o_sb, in0=o_sb, in1=xres)
                        nc.sync.dma_start(
                            out=out_dram[128 * m:128 * (m + 1), gcols], in_=o_sb)

    nc.compile()
    return nc


_NC_CACHE = None


def _prep_inputs(inputs):
    x = np.asarray(inputs["x"], np.float32)

    def tile4(v):
        return np.asarray(v, np.float32).reshape(4, 128).T

    biases = np.concatenate(
        [tile4(inputs[k]) for k in ("bq", "bk", "bv", "bo")], axis=1)
    gb = np.concatenate(
        [tile4(inputs["gn_gamma"]), tile4(inputs["gn_beta"])], axis=1)
    def split16(w):
        hi = w.astype(np.float16)
        lo = (w - hi.astype(np.float32)).astype(np.float16)
        return hi, lo

    wqT = np.ascontiguousarray(np.asarray(inputs["wq"], np.float32).T)
    wkT = np.ascontiguousarray(np.asarray(inputs["wk"], np.float32).T)
    wqh, wql = split16(wqT)
    wkh, wkl = split16(wkT)
    shared = {
        "wqTh": wqh, "wqTl": wql,
        "wkTh": wkh, "wkTl": wkl,
        "wvT": np.ascontiguousarray(np.asarray(inputs["wv"], np.float32).T),
        "woT": np.ascontiguousarray(np.asarray(inputs["wo"], np.float32).T),
        "biases": np.ascontiguousarray(biases),
        "gammabeta": np.ascontiguousarray(gb),
        "bias_rows": np.ascontiguousarray(np.stack(
            [np.asarray(inputs["bq"], np.float32),
             np.asarray(inputs["bk"], np.float32)])),
    }
    return [dict(shared, x=np.ascontiguousarray(x[i].reshape(C, HW)))
            for i in range(x.shape[0])]


def kernel(**inputs):
    global _NC_CACHE
    if _NC_CACHE is None:
        _NC_CACHE = build()
    nc = _NC_CACHE
    x = np.asarray(inputs["x"], np.float32)
    b, c, h, w = x.shape
    in_maps = _prep_inputs(inputs)
    res = run_bass_kernel_spmd(nc, in_maps, list(range(b)))
    out = np.stack([res.results[i]["out"].reshape(c, h, w) for i in range(b)])
    return out.astype(np.float32)


if __name__ == "__main__":
    import time
    t0 = time.time()
    build()
    print(f"build ok in {time.time()-t0:.1f}s")



# revision 6
# speedup vs baseline: 1.7964x; 1.7964x over previous
"""NonLocalBlock (GroupNorm + single-head 4096x4096 attention + residual)
Trainium2 Bass kernel, data-parallel over batch: 1 image per NeuronCore x8.

Per image (x: [512, 4096] channels-major):
  pass0: GroupNorm stats (bn_stats per channel, group-combine via tiny matmuls)
  passA: per hw-chunk of 512: normalize -> h (fp32r), h16 = fp16(h) resident,
         kk = (Wq^T Wk) @ h via fp32r 1-pass (M precomputed on host in fp64),
         kk16 resident, vT (fp32r 1-pass) -> fp16 resident.
  attention: logits = h16^T @ kk16 (1-pass fp16 matmuls straight into PSUM,
         all 8 chunks of a q-tile live in PSUM banks), row max on DVE from
         PSUM, ACT exp reads PSUM -> fp16 probs (+exact fp32 row sums),
         fp16 PE-transpose probs, fp16 attn@v, fp16 transpose attn_h (+bv),
         fp16 output projection, +bo +residual fused on DVE, store.
  q is never materialized: softmax(q^T k) == softmax(h^T (Wq^T Wk) h); the
  q-row bias term q^T bk is softmax-invariant; bq column term handled in the
  with_qk_bias build variant (bq/bk are zero in practice).
"""
import sys

sys.path.insert(0, '/opt/trn_rl_repo')
import numpy as np
import concourse.bass as bass
import concourse.bacc as bacc
import concourse.mybir as mybir
import concourse.tile as tile
from concourse.bass_utils import run_bass_kernel_spmd

F32 = mybir.dt.float32
F32R = mybir.dt.float32r
F16 = mybir.dt.float16
AF = mybir.ActivationFunctionType
AX = mybir.AxisListType
OP = mybir.AluOpType

C = 512
HW = 4096
NT = 4            # channel tiles of 128
NCH = 8           # hw chunks of 512
NQT = 32          # q tiles of 128
GSIZE = 16        # channels per group
EPS = 1e-5
SCALE = float(np.float32(512.0) ** 0.5)


def build(with_qk_bias=False):
    nc = bacc.Bacc('TRN2', target_bir_lowering=False, debug=False)

    x_in = nc.declare_dram_parameter("x", [C, HW], F32, isOutput=False)
    mT_in = nc.declare_dram_parameter("mT", [C, C], F32, isOutput=False)
    wvT_in = nc.declare_dram_parameter("wvT", [C, C], F32, isOutput=False)
    wo16_in = nc.declare_dram_parameter("woT16", [C, C], F16, isOutput=False)
    bias_in = nc.declare_dram_parameter("biases", [128, 16], F32,
                                        isOutput=False)  # bq|bk|bv|bo as [128,4]
    gb_in = nc.declare_dram_parameter("gammabeta", [128, 8], F32,
                                      isOutput=False)  # gamma|beta as [128,4]
    if with_qk_bias:
        u_in = nc.declare_dram_parameter("uT", [C, 1], F32, isOutput=False)
    out_dram = nc.declare_dram_parameter("out", [C, HW], F32, isOutput=True)

    a16 = np.zeros((128, 8), np.float32)
    for p in range(128):
        a16[p, p // GSIZE] = 1.0 / GSIZE
    b8 = np.zeros((8, 128), np.float32)
    for p in range(128):
        b8[p // GSIZE, p] = 1.0
    a16_d = nc.inline_tensor(a16, "a16")
    b8_d = nc.inline_tensor(b8, "b8")
    ident_d = nc.inline_tensor(np.eye(128, dtype=np.float32), "ident128")
    id16_d = nc.inline_tensor(np.eye(128, dtype=np.float16), "ident16")

    with tile.TileContext(nc) as tc:
        with (
            tc.tile_pool(name="res", bufs=1) as res,
            tc.tile_pool(name="pp_log", bufs=3, space="PSUM") as pp_log,
            tc.tile_pool(name="pp_t", bufs=2, space="PSUM") as pp_t,
            tc.tile_pool(name="pp_at", bufs=1, space="PSUM") as pp_at,
            tc.tile_pool(name="pp_o", bufs=2, space="PSUM") as pp_o,
        ):
            # ---------- residents ----------
            h16_res = [res.tile([128, HW], F16, tag=f"h16_{t}", name=f"h16_{t}")
                       for t in range(NT)]
            kk16_res = [res.tile([128, HW], F16, tag=f"kk{t}", name=f"kk{t}")
                        for t in range(NT)]
            vT_res = [res.tile([128, C], F16, tag=f"vT{m}", name=f"vT{m}")
                      for m in range(NQT)]
            mT_sb = [res.tile([128, C], F32R, tag=f"mT{t}", name=f"mT{t}")
                     for t in range(NT)]
            wv_sb = [res.tile([128, C], F32R, tag=f"wv{t}", name=f"wv{t}")
                     for t in range(NT)]
            wo_sb = [res.tile([128, C], F16, tag=f"wo{t}", name=f"wo{t}")
                     for t in range(NT)]
            for t in range(NT):
                sl = slice(128 * t, 128 * (t + 1))
                nc.gpsimd.dma_start(out=mT_sb[t], in_=mT_in[sl, :])
                nc.gpsimd.dma_start(out=wv_sb[t], in_=wvT_in[sl, :])
                nc.sync.dma_start(out=wo_sb[t], in_=wo16_in[sl, :])
            biases = res.tile([128, 16], F32, tag="biases")
            nc.sync.dma_start(out=biases, in_=bias_in[:])
            bv = biases[:, 8:12]
            bo = biases[:, 12:16]
            gmbt = res.tile([128, 8], F32, tag="gmbt")
            nc.sync.dma_start(out=gmbt, in_=gb_in[:])
            gam = gmbt[:, 0:4]
            bet = gmbt[:, 4:8]
            a16_sb = res.tile([128, 8], F32, tag="a16")
            nc.sync.dma_start(out=a16_sb, in_=a16_d[:])
            b8_sb = res.tile([8, 128], F32, tag="b8")
            nc.sync.dma_start(out=b8_sb, in_=b8_d[:])
            id_sb = res.tile([128, 128], F32, tag="ident")
            nc.sync.dma_start(out=id_sb, in_=ident_d[:])
            id16_sb = res.tile([128, 128], F16, tag="ident16")
            nc.sync.dma_start(out=id16_sb, in_=id16_d[:])
            if with_qk_bias:
                u_sb = res.tile([128, NT], F32R, tag="u_sb")
                nc.gpsimd.dma_start(
                    out=u_sb, in_=u_in[:].rearrange("(t p) o -> p (t o)", p=128))
                ones_col = res.tile([1, 128], F32R, tag="ones_col")
                nc.vector.memset(ones_col, 1.0)
            eps8 = res.tile([8, 1], F32, tag="eps8")
            nc.vector.memset(eps8, EPS)
            scale_sb = res.tile([128, NT], F32, tag="scale")
            shift_sb = res.tile([128, NT], F32, tag="shift")

            # PE warmup: ~5us of dummy transposes while pass0 stats run,
            # so HAM unthrottles (1.2->2.4GHz) before passA matmuls start.
            wps = pp_log.tile([128, 128], F32, tag="ps_l", name="wps")
            for _ in range(24):
                nc.tensor.transpose(wps, id_sb, id_sb)

            # ---------- pass 0: GroupNorm statistics ----------
            with tc.tile_pool(name="p0", bufs=4) as p0, \
                 tc.tile_pool(name="p0s", bufs=1) as p0s:
                st6 = p0s.tile([128, NT, NCH, 6], F32, tag="st6")
                for n in range(NCH):
                    for t in range(NT):
                        xc = p0.tile([128, 512], F32, tag="x0")
                        nc.sync.dma_start(
                            out=xc,
                            in_=x_in[128 * t:128 * (t + 1), 512 * n:512 * (n + 1)])
                        nc.vector.bn_stats(out=st6[:, t, n, :], in_=xc)
                mv = p0s.tile([128, NT, 2], F32, tag="mv")
                for t in range(NT):
                    nc.vector.bn_aggr(out=mv[:, t, :], in_=st6[:, t, :, :])
                # stats_in: cols 0-3 mean_t, cols 4-7 E[x^2]_t
                stats_in = p0s.tile([128, 8], F32, tag="stats_in")
                for t in range(NT):
                    nc.vector.tensor_copy(stats_in[:, t:t + 1], mv[:, t, 0:1])
                    nc.vector.tensor_mul(stats_in[:, 4 + t:5 + t],
                                         mv[:, t, 0:1], mv[:, t, 0:1])
                    nc.vector.tensor_add(stats_in[:, 4 + t:5 + t],
                                         stats_in[:, 4 + t:5 + t], mv[:, t, 1:2])
                ps_g = pp_o.tile([8, 8], F32, tag="ps_o")
                nc.tensor.matmul(ps_g, a16_sb, stats_in, start=True, stop=True)
                g_sb = p0s.tile([8, 8], F32, tag="g_sb")
                nc.vector.tensor_copy(g_sb, ps_g)
                # group var = E[x^2]_g - mean_g^2 ; rstd = exp(-0.5*ln(var+eps))
                var_g = p0s.tile([8, 4], F32, tag="var_g")
                nc.vector.tensor_mul(var_g, g_sb[:, 0:4], g_sb[:, 0:4])
                nc.vector.tensor_tensor(out=var_g, in0=g_sb[:, 4:8], in1=var_g,
                                        op=OP.subtract)
                bc_in = p0s.tile([8, 8], F32, tag="bc_in")
                nc.vector.tensor_copy(bc_in[:, 0:4], g_sb[:, 0:4])
                nc.scalar.activation(out=bc_in[:, 4:8], in_=var_g, func=AF.Ln,
                                     bias=eps8, scale=1.0)
                nc.scalar.activation(out=bc_in[:, 4:8], in_=bc_in[:, 4:8],
                                     func=AF.Exp, bias=0.0, scale=-0.5)
                ps_bc = pp_o.tile([128, 8], F32, tag="ps_o")
                nc.tensor.matmul(ps_bc, b8_sb, bc_in, start=True, stop=True)
                chan = p0s.tile([128, 8], F32, tag="chan")
                nc.vector.tensor_copy(chan, ps_bc)
                # scale = gamma * rstd ; shift = beta - mean*scale
                nc.vector.tensor_mul(scale_sb, gam, chan[:, 4:8])
                tmp = p0s.tile([128, NT], F32, tag="tmp")
                nc.vector.tensor_mul(tmp, chan[:, 0:4], scale_sb)
                nc.vector.tensor_tensor(out=shift_sb, in0=bet, in1=tmp,
                                        op=OP.subtract)

            # ---------- pass A: hidden -> h16, kk16, vT16 (+u row) ----------
            with tc.tile_pool(name="pa_x", bufs=3) as pa_x, \
                 tc.tile_pool(name="pa_hr", bufs=8) as pa_hr:
                if with_qk_bias:
                    r_row = res.tile([1, HW], F32, tag="r_row")
                for n in range(NCH):
                    cols = slice(512 * n, 512 * (n + 1))
                    hid_r = []
                    for t in range(NT):
                        xc = pa_x.tile([128, 512], F32, tag="xA")
                        nc.sync.dma_start(
                            out=xc, in_=x_in[128 * t:128 * (t + 1), cols])
                        hr = pa_hr.tile([128, 512], F32R, tag="hid_r", bufs=4)
                        nc.vector.tensor_scalar(
                            out=hr, in0=xc,
                            scalar1=scale_sb[:, t:t + 1],
                            scalar2=shift_sb[:, t:t + 1],
                            op0=OP.mult, op1=OP.add)
                        hid_r.append(hr)
                        nc.scalar.copy(out=h16_res[t][:, cols], in_=hr)
                    # kk = M @ h (fp32r 1-pass), round to fp16
                    for t in range(NT):
                        ps = pp_o.tile([128, 512], F32, tag="ps_o")
                        for kc in range(NT):
                            nc.tensor.matmul(
                                ps, mT_sb[kc][:, 128 * t:128 * (t + 1)],
                                hid_r[kc], start=(kc == 0), stop=(kc == 3))
                        nc.scalar.copy(out=kk16_res[t][:, cols], in_=ps)
                    # vT (fp32r 1-pass): out[hw_t 128, c 512], round to fp16
                    # (bv folded into attn_h later: softmax weights sum to 1)
                    for t in range(NT):
                        ps = pp_o.tile([128, 512], F32, tag="ps_o")
                        for kc in range(NT):
                            nc.tensor.matmul(
                                ps, hid_r[kc][:, 128 * t:128 * (t + 1)],
                                wv_sb[kc], start=(kc == 0), stop=(kc == 3))
                        nc.vector.tensor_copy(vT_res[4 * n + t], ps)
                    if with_qk_bias:
                        # r = u^T h  [1, 512] chunk (bq column term)
                        ps_r = pp_log.tile([1, 512], F32, tag="ps_l")
                        for kc in range(NT):
                            nc.tensor.matmul(
                                ps_r, u_sb[:, kc:kc + 1], hid_r[kc],
                                start=(kc == 0), stop=(kc == 3))
                        nc.vector.tensor_copy(r_row[:, cols], ps_r)

            # ---------- attention ----------
            with tc.tile_pool(name="at_l", bufs=2) as at_l, \
                 tc.tile_pool(name="at_p", bufs=3) as at_p, \
                 tc.tile_pool(name="at_pt", bufs=3) as at_pt, \
                 tc.tile_pool(name="at_s", bufs=2) as at_s, \
                 tc.tile_pool(name="at_h4", bufs=2) as at_h4, \
                 tc.tile_pool(name="at_o", bufs=2) as at_o:
                for g in range(NCH):
                    attnh4 = at_h4.tile([128, NT, 512], F16, tag="attnh4")
                    for qq in range(4):
                        qt = 4 * g + qq
                        qcols = slice(128 * qt, 128 * (qt + 1))
                        # logits per chunk -> PSUM; row max on DVE; stage to
                        # SBUF fp32 (copies alternate ACT/GpSimd), free bank
                        lg = at_l.tile([128, HW], F32, tag="lg")
                        maxs = at_s.tile([128, NCH], F32, tag="maxs")
                        for n in range(NCH):
                            ncols = slice(512 * n, 512 * (n + 1))
                            ps_l = pp_log.tile([128, 512], F32, tag="ps_l")
                            for kc in range(NT):
                                nc.tensor.matmul(
                                    ps_l, h16_res[kc][:, qcols],
                                    kk16_res[kc][:, ncols],
                                    start=(kc == 0), stop=(kc == 3 and
                                                           not with_qk_bias))
                            if with_qk_bias:
                                nc.tensor.matmul(
                                    ps_l, ones_col, r_row[:, ncols],
                                    start=False, stop=True)
                            nc.vector.reduce_max(out=maxs[:, n:n + 1], in_=ps_l,
                                                 axis=AX.X)
                            if n % 2 == 0:
                                nc.scalar.copy(out=lg[:, ncols], in_=ps_l)
                            else:
                                nc.vector.tensor_copy(out=lg[:, ncols],
                                                      in_=ps_l)
                        negmax = at_s.tile([128, 1], F32, tag="negmax")
                        nc.vector.reduce_max(out=negmax, in_=maxs, axis=AX.X,
                                             negate=True)
                        negmax_s = at_s.tile([128, 1], F32, tag="negmax_s")
                        nc.vector.tensor_scalar_mul(out=negmax_s, in0=negmax,
                                                    scalar1=SCALE)
                        sums = at_s.tile([128, NCH], F32, tag="sums")
                        ps_at = pp_at.tile([128, C], F32, tag="ps_at")
                        for n in range(NCH):
                            probs = at_p.tile([128, 512], F16, tag="probs")
                            nc.scalar.activation(
                                out=probs, in_=lg[:, 512 * n:512 * (n + 1)],
                                func=AF.Exp, bias=negmax_s, scale=SCALE,
                                accum_out=sums[:, n:n + 1])
                            ps_t = pp_t.tile([128, 512], F16, tag="ps_t")
                            for j in range(4):
                                nc.tensor.transpose(
                                    ps_t[:, 128 * j:128 * (j + 1)],
                                    probs[:, 128 * j:128 * (j + 1)], id16_sb)
                            pT = at_pt.tile([128, 512], F16, tag="pT")
                            nc.vector.tensor_copy(pT, ps_t)
                            for j in range(4):
                                nc.tensor.matmul(
                                    ps_at, pT[:, 128 * j:128 * (j + 1)],
                                    vT_res[4 * n + j],
                                    start=(n == 0 and j == 0),
                                    stop=(n == 7 and j == 3))
                        rowsum = at_s.tile([128, 1], F32, tag="rowsum")
                        nc.vector.reduce_sum(out=rowsum, in_=sums, axis=AX.X)
                        rinv = at_s.tile([128, 1], F32, tag="rinv")
                        nc.vector.reciprocal(out=rinv, in_=rowsum)
                        attn = at_s.tile([128, C], F16, tag="attn")
                        nc.vector.tensor_scalar_mul(out=attn, in0=ps_at,
                                                    scalar1=rinv)
                        # transpose attn [q, c] -> attn_h [c, q]; add bv
                        ps_t2 = pp_t.tile([128, 512], F16, tag="ps_t")
                        for i in range(NT):
                            nc.tensor.transpose(
                                ps_t2[:, 128 * i:128 * (i + 1)],
                                attn[:, 128 * i:128 * (i + 1)], id16_sb)
                        for i in range(NT):
                            nc.vector.tensor_scalar_add(
                                out=attnh4[:, i, 128 * qq:128 * (qq + 1)],
                                in0=ps_t2[:, 128 * i:128 * (i + 1)],
                                scalar1=bv[:, i:i + 1])
                    # batched output projection (fp16, N=512) + bias + residual
                    gcols = slice(512 * g, 512 * (g + 1))
                    for m in range(NT):
                        ps_o = pp_o.tile([128, 512], F32, tag="ps_o")
                        for kc in range(NT):
                            nc.tensor.matmul(
                                ps_o, wo_sb[kc][:, 128 * m:128 * (m + 1)],
                                attnh4[:, kc, :], start=(kc == 0), stop=(kc == 3))
                        xres = at_o.tile([128, 512], F32, tag="xres")
                        nc.sync.dma_start(
                            out=xres, in_=x_in[128 * m:128 * (m + 1), gcols])
                        o_sb = at_o.tile([128, 512], F32, tag="o_sb")
                        nc.vector.scalar_tensor_tensor(
                            out=o_sb, in0=ps_o, scalar=bo[:, m:m + 1],
                            in1=xres, op0=OP.add, op1=OP.add)
                        nc.sync.dma_start(
                            out=out_dram[128 * m:128 * (m + 1), gcols], in_=o_sb)

    nc.compile()
    return nc


_NC_CACHE = None
_NC_BIAS_CACHE = None


def _prep_inputs(inputs):
    x = np.asarray(inputs["x"], np.float32)

    def tile4(v):
        return np.asarray(v, np.float32).reshape(4, 128).T

    biases = np.concatenate(
        [tile4(inputs[k]) for k in ("bq", "bk", "bv", "bo")], axis=1)
    gb = np.concatenate(
        [tile4(inputs["gn_gamma"]), tile4(inputs["gn_beta"])], axis=1)
    wq = np.asarray(inputs["wq"], np.float64)
    wk = np.asarray(inputs["wk"], np.float64)
    mT = np.ascontiguousarray((wk.T @ wq).astype(np.float32))
    shared = {
        "mT": mT,
        "wvT": np.ascontiguousarray(np.asarray(inputs["wv"], np.float32).T),
        "woT16": np.ascontiguousarray(
            np.asarray(inputs["wo"], np.float32).T.astype(np.float16)),
        "biases": np.ascontiguousarray(biases),
        "gammabeta": np.ascontiguousarray(gb),
    }
    if np.any(np.asarray(inputs["bq"], np.float32)) or \
       np.any(np.asarray(inputs["bk"], np.float32)):
        u = wk.T @ np.asarray(inputs["bq"], np.float64)
        shared["uT"] = np.ascontiguousarray(
            u.astype(np.float32).reshape(C, 1))
    return [dict(shared, x=np.ascontiguousarray(x[i].reshape(C, HW)))
            for i in range(x.shape[0])]


def kernel(**inputs):
    global _NC_CACHE, _NC_BIAS_CACHE
    x = np.asarray(inputs["x"], np.float32)
    b, c, h, w = x.shape
    in_maps = _prep_inputs(inputs)
    if "uT" in in_maps[0]:
        if _NC_BIAS_CACHE is None:
            _NC_BIAS_CACHE = build(with_qk_bias=True)
        nc = _NC_BIAS_CACHE
    else:
        if _NC_CACHE is None:
            _NC_CACHE = build(with_qk_bias=False)
        nc = _NC_CACHE
    res = run_bass_kernel_spmd(nc, in_maps, list(range(b)))
    out = np.stack([res.results[i]["out"].reshape(c, h, w) for i in range(b)])
    return out.astype(np.float32)


if __name__ == "__main__":
    import time
    t0 = time.time()
    build()
    print(f"build ok in {time.time()-t0:.1f}s")


# revision 9
# speedup vs baseline: 1.8694x; 1.0407x over previous
"""NonLocalBlock (GroupNorm + single-head 4096x4096 attention + residual)
Trainium2 Bass kernel, data-parallel over batch: 1 image per NeuronCore x8.

Per image (x: [512, 4096] channels-major):
  pass0: GroupNorm stats (bn_stats per channel, group-combine via tiny matmuls)
  passA: per hw-chunk of 512: normalize -> h (fp32r), h16 = fp16(h) resident,
         kk = (Wq^T Wk) @ h via fp32r 1-pass (M precomputed on host in fp64),
         kk16 resident, vT (fp32r 1-pass) -> fp16 resident.
  attention: logits = h16^T @ kk16 (1-pass fp16 matmuls straight into PSUM,
         all 8 chunks of a q-tile live in PSUM banks), row max on DVE from
         PSUM, ACT exp reads PSUM -> fp16 probs (+exact fp32 row sums),
         fp16 PE-transpose probs, fp16 attn@v, fp16 transpose attn_h (+bv),
         fp16 output projection, +bo +residual fused on DVE, store.
  q is never materialized: softmax(q^T k) == softmax(h^T (Wq^T Wk) h); the
  q-row bias term q^T bk is softmax-invariant; bq column term handled in the
  with_qk_bias build variant (bq/bk are zero in practice).
"""
import sys

sys.path.insert(0, '/opt/trn_rl_repo')
import numpy as np
import concourse.bass as bass
import concourse.bacc as bacc
import concourse.mybir as mybir
import concourse.tile as tile
from concourse.bass_utils import run_bass_kernel_spmd

F32 = mybir.dt.float32
F32R = mybir.dt.float32r
F16 = mybir.dt.float16
AF = mybir.ActivationFunctionType
AX = mybir.AxisListType
OP = mybir.AluOpType

C = 512
HW = 4096
NT = 4            # channel tiles of 128
NCH = 8           # hw chunks of 512
NQT = 32          # q tiles of 128
GSIZE = 16        # channels per group
EPS = 1e-5
SCALE = float(np.float32(512.0) ** 0.5)


def build(with_qk_bias=False):
    nc = bacc.Bacc('TRN2', target_bir_lowering=False, debug=False)

    x_in = nc.declare_dram_parameter("x", [C, HW], F32, isOutput=False)
    mT_in = nc.declare_dram_parameter("mT", [C, C], F32, isOutput=False)
    wvT_in = nc.declare_dram_parameter("wvT", [C, C], F32, isOutput=False)
    wo16_in = nc.declare_dram_parameter("woT16", [C, C], F16, isOutput=False)
    bias_in = nc.declare_dram_parameter("biases", [128, 16], F32,
                                        isOutput=False)  # bq|bk|bv|bo as [128,4]
    gb_in = nc.declare_dram_parameter("gammabeta", [128, 8], F32,
                                      isOutput=False)  # gamma|beta as [128,4]
    if with_qk_bias:
        u_in = nc.declare_dram_parameter("uT", [C, 1], F32, isOutput=False)
    out_dram = nc.declare_dram_parameter("out", [C, HW], F32, isOutput=True)

    a16 = np.zeros((128, 8), np.float32)
    for p in range(128):
        a16[p, p // GSIZE] = 1.0 / GSIZE
    b8 = np.zeros((8, 128), np.float32)
    for p in range(128):
        b8[p // GSIZE, p] = 1.0
    a16_d = nc.inline_tensor(a16, "a16")
    b8_d = nc.inline_tensor(b8, "b8")
    ident_d = nc.inline_tensor(np.eye(128, dtype=np.float32), "ident128")
    id16_d = nc.inline_tensor(np.eye(128, dtype=np.float16), "ident16")

    with tile.TileContext(nc) as tc:
        with (
            tc.tile_pool(name="res", bufs=1) as res,
            tc.tile_pool(name="pp_log", bufs=3, space="PSUM") as pp_log,
            tc.tile_pool(name="pp_t", bufs=2, space="PSUM") as pp_t,
            tc.tile_pool(name="pp_at", bufs=1, space="PSUM") as pp_at,
            tc.tile_pool(name="pp_o", bufs=2, space="PSUM") as pp_o,
        ):
            # ---------- residents ----------
            h16_res = [res.tile([128, HW], F16, tag=f"h16_{t}", name=f"h16_{t}")
                       for t in range(NT)]
            kk16_res = [res.tile([128, HW], F16, tag=f"kk{t}", name=f"kk{t}")
                        for t in range(NT)]
            vT_res = [res.tile([128, C], F16, tag=f"vT{m}", name=f"vT{m}")
                      for m in range(NQT)]
            mT_sb = [res.tile([128, C], F32R, tag=f"mT{t}", name=f"mT{t}")
                     for t in range(NT)]
            wv_sb = [res.tile([128, C], F32R, tag=f"wv{t}", name=f"wv{t}")
                     for t in range(NT)]
            wo_sb = [res.tile([128, C], F16, tag=f"wo{t}", name=f"wo{t}")
                     for t in range(NT)]
            for t in range(NT):
                sl = slice(128 * t, 128 * (t + 1))
                nc.gpsimd.dma_start(out=mT_sb[t], in_=mT_in[sl, :])
                nc.gpsimd.dma_start(out=wv_sb[t], in_=wvT_in[sl, :])
                nc.sync.dma_start(out=wo_sb[t], in_=wo16_in[sl, :])
            biases = res.tile([128, 16], F32, tag="biases")
            nc.sync.dma_start(out=biases, in_=bias_in[:])
            bv = biases[:, 8:12]
            bo = biases[:, 12:16]
            gmbt = res.tile([128, 8], F32, tag="gmbt")
            nc.sync.dma_start(out=gmbt, in_=gb_in[:])
            gam = gmbt[:, 0:4]
            bet = gmbt[:, 4:8]
            a16_sb = res.tile([128, 8], F32, tag="a16")
            nc.sync.dma_start(out=a16_sb, in_=a16_d[:])
            b8_sb = res.tile([8, 128], F32, tag="b8")
            nc.sync.dma_start(out=b8_sb, in_=b8_d[:])
            id_sb = res.tile([128, 128], F32, tag="ident")
            nc.sync.dma_start(out=id_sb, in_=ident_d[:])
            id16_sb = res.tile([128, 128], F16, tag="ident16")
            nc.sync.dma_start(out=id16_sb, in_=id16_d[:])
            if with_qk_bias:
                u_sb = res.tile([128, NT], F32R, tag="u_sb")
                nc.gpsimd.dma_start(
                    out=u_sb, in_=u_in[:].rearrange("(t p) o -> p (t o)", p=128))
                ones_col = res.tile([1, 128], F32R, tag="ones_col")
                nc.vector.memset(ones_col, 1.0)
            eps8 = res.tile([8, 1], F32, tag="eps8")
            nc.vector.memset(eps8, EPS)
            scale_sb = res.tile([128, NT], F32, tag="scale")
            shift_sb = res.tile([128, NT], F32, tag="shift")

            # PE warmup: dummy transposes interleaved through pass0 so HAM
            # stays unthrottled (1.2->2.4GHz) until passA matmuls start.
            wps = pp_log.tile([128, 128], F32, tag="ps_l", name="wps")
            for _ in range(24):
                nc.tensor.transpose(wps, id_sb, id_sb)

            # ---------- pass 0: GroupNorm statistics ----------
            with tc.tile_pool(name="p0", bufs=4) as p0, \
                 tc.tile_pool(name="p0s", bufs=1) as p0s:
                st6 = p0s.tile([128, NT, NCH, 6], F32, tag="st6")
                for n in range(NCH):
                    for t in range(NT):
                        xc = p0.tile([128, 512], F32, tag="x0")
                        nc.sync.dma_start(
                            out=xc,
                            in_=x_in[128 * t:128 * (t + 1), 512 * n:512 * (n + 1)])
                        nc.vector.bn_stats(out=st6[:, t, n, :], in_=xc)
                        # keep-warm: depends on xc's DMA, so it lands mid-pass0
                        for _ in range(2):
                            nc.tensor.transpose(wps, xc[:, 0:128], id_sb)
                mv = p0s.tile([128, NT, 2], F32, tag="mv")
                for t in range(NT):
                    nc.vector.bn_aggr(out=mv[:, t, :], in_=st6[:, t, :, :])
                # stats_in: cols 0-3 mean_t, cols 4-7 E[x^2]_t
                stats_in = p0s.tile([128, 8], F32, tag="stats_in")
                for t in range(NT):
                    nc.vector.tensor_copy(stats_in[:, t:t + 1], mv[:, t, 0:1])
                    nc.vector.tensor_mul(stats_in[:, 4 + t:5 + t],
                                         mv[:, t, 0:1], mv[:, t, 0:1])
                    nc.vector.tensor_add(stats_in[:, 4 + t:5 + t],
                                         stats_in[:, 4 + t:5 + t], mv[:, t, 1:2])
                ps_g = pp_o.tile([8, 8], F32, tag="ps_o")
                nc.tensor.matmul(ps_g, a16_sb, stats_in, start=True, stop=True)
                g_sb = p0s.tile([8, 8], F32, tag="g_sb")
                nc.vector.tensor_copy(g_sb, ps_g)
                # group var = E[x^2]_g - mean_g^2 ; rstd = exp(-0.5*ln(var+eps))
                var_g = p0s.tile([8, 4], F32, tag="var_g")
                nc.vector.tensor_mul(var_g, g_sb[:, 0:4], g_sb[:, 0:4])
                nc.vector.tensor_tensor(out=var_g, in0=g_sb[:, 4:8], in1=var_g,
                                        op=OP.subtract)
                bc_in = p0s.tile([8, 8], F32, tag="bc_in")
                nc.vector.tensor_copy(bc_in[:, 0:4], g_sb[:, 0:4])
                nc.scalar.activation(out=bc_in[:, 4:8], in_=var_g, func=AF.Ln,
                                     bias=eps8, scale=1.0)
                nc.scalar.activation(out=bc_in[:, 4:8], in_=bc_in[:, 4:8],
                                     func=AF.Exp, bias=0.0, scale=-0.5)
                ps_bc = pp_o.tile([128, 8], F32, tag="ps_o")
                nc.tensor.matmul(ps_bc, b8_sb, bc_in, start=True, stop=True)
                chan = p0s.tile([128, 8], F32, tag="chan")
                nc.vector.tensor_copy(chan, ps_bc)
                # scale = gamma * rstd ; shift = beta - mean*scale
                nc.vector.tensor_mul(scale_sb, gam, chan[:, 4:8])
                tmp = p0s.tile([128, NT], F32, tag="tmp")
                nc.vector.tensor_mul(tmp, chan[:, 0:4], scale_sb)
                nc.vector.tensor_tensor(out=shift_sb, in0=bet, in1=tmp,
                                        op=OP.subtract)

            # ---------- pass A: hidden -> h16, kk16, vT16 (+u row) ----------
            with tc.tile_pool(name="pa_x", bufs=3) as pa_x, \
                 tc.tile_pool(name="pa_hr", bufs=8) as pa_hr:
                if with_qk_bias:
                    r_row = res.tile([1, HW], F32, tag="r_row")
                for n in range(NCH):
                    cols = slice(512 * n, 512 * (n + 1))
                    hid_r = []
                    for t in range(NT):
                        xc = pa_x.tile([128, 512], F32, tag="xA")
                        nc.sync.dma_start(
                            out=xc, in_=x_in[128 * t:128 * (t + 1), cols])
                        hr = pa_hr.tile([128, 512], F32R, tag="hid_r", bufs=8)
                        nc.vector.tensor_scalar(
                            out=hr, in0=xc,
                            scalar1=scale_sb[:, t:t + 1],
                            scalar2=shift_sb[:, t:t + 1],
                            op0=OP.mult, op1=OP.add)
                        hid_r.append(hr)
                        nc.scalar.copy(out=h16_res[t][:, cols], in_=hr)
                    # kk = M @ h (fp32r 1-pass), round to fp16
                    for t in range(NT):
                        ps = pp_o.tile([128, 512], F32, tag="ps_o")
                        for kc in range(NT):
                            nc.tensor.matmul(
                                ps, mT_sb[kc][:, 128 * t:128 * (t + 1)],
                                hid_r[kc], start=(kc == 0), stop=(kc == 3))
                        nc.scalar.copy(out=kk16_res[t][:, cols], in_=ps)
                    # vT (fp32r 1-pass): out[hw_t 128, c 512], round to fp16
                    # (bv folded into attn_h later: softmax weights sum to 1)
                    for t in range(NT):
                        ps = pp_o.tile([128, 512], F32, tag="ps_o")
                        for kc in range(NT):
                            nc.tensor.matmul(
                                ps, hid_r[kc][:, 128 * t:128 * (t + 1)],
                                wv_sb[kc], start=(kc == 0), stop=(kc == 3))
                        nc.vector.tensor_copy(vT_res[4 * n + t], ps)
                    if with_qk_bias:
                        # r = u^T h  [1, 512] chunk (bq column term)
                        ps_r = pp_log.tile([1, 512], F32, tag="ps_l")
                        for kc in range(NT):
                            nc.tensor.matmul(
                                ps_r, u_sb[:, kc:kc + 1], hid_r[kc],
                                start=(kc == 0), stop=(kc == 3))
                        nc.vector.tensor_copy(r_row[:, cols], ps_r)

            # ---------- attention ----------
            with tc.tile_pool(name="at_l", bufs=2) as at_l, \
                 tc.tile_pool(name="at_p", bufs=3) as at_p, \
                 tc.tile_pool(name="at_pt", bufs=3) as at_pt, \
                 tc.tile_pool(name="at_s", bufs=2) as at_s, \
                 tc.tile_pool(name="at_h4", bufs=2) as at_h4, \
                 tc.tile_pool(name="at_o", bufs=2) as at_o:
                for g in range(NCH):
                    attnh4 = at_h4.tile([128, NT, 512], F16, tag="attnh4")
                    for qq in range(4):
                        qt = 4 * g + qq
                        qcols = slice(128 * qt, 128 * (qt + 1))
                        # logits per chunk -> PSUM; row max on DVE; stage to
                        # SBUF fp32 (copies alternate ACT/GpSimd), free bank
                        lg = at_l.tile([128, HW], F32, tag="lg")
                        maxs = at_s.tile([128, NCH], F32, tag="maxs")
                        for n in range(NCH):
                            ncols = slice(512 * n, 512 * (n + 1))
                            ps_l = pp_log.tile([128, 512], F32, tag="ps_l")
                            for kc in range(NT):
                                nc.tensor.matmul(
                                    ps_l, h16_res[kc][:, qcols],
                                    kk16_res[kc][:, ncols],
                                    start=(kc == 0), stop=(kc == 3 and
                                                           not with_qk_bias))
                            if with_qk_bias:
                                nc.tensor.matmul(
                                    ps_l, ones_col, r_row[:, ncols],
                                    start=False, stop=True)
                            nc.vector.reduce_max(out=maxs[:, n:n + 1], in_=ps_l,
                                                 axis=AX.X)
                            if n % 2 == 0:
                                nc.scalar.copy(out=lg[:, ncols], in_=ps_l)
                            else:
                                nc.vector.tensor_copy(out=lg[:, ncols],
                                                      in_=ps_l)
                        negmax = at_s.tile([128, 1], F32, tag="negmax")
                        nc.vector.reduce_max(out=negmax, in_=maxs, axis=AX.X,
                                             negate=True)
                        negmax_s = at_s.tile([128, 1], F32, tag="negmax_s")
                        nc.vector.tensor_scalar_mul(out=negmax_s, in0=negmax,
                                                    scalar1=SCALE)
                        sums = at_s.tile([128, NCH], F32, tag="sums")
                        ps_at = pp_at.tile([128, C], F32, tag="ps_at")
                        for n in range(NCH):
                            probs = at_p.tile([128, 512], F16, tag="probs")
                            nc.scalar.activation(
                                out=probs, in_=lg[:, 512 * n:512 * (n + 1)],
                                func=AF.Exp, bias=negmax_s, scale=SCALE,
                                accum_out=sums[:, n:n + 1])
                            ps_t = pp_t.tile([128, 512], F16, tag="ps_t")
                            for j in range(4):
                                nc.tensor.transpose(
                                    ps_t[:, 128 * j:128 * (j + 1)],
                                    probs[:, 128 * j:128 * (j + 1)], id16_sb)
                            pT = at_pt.tile([128, 512], F16, tag="pT")
                            nc.vector.tensor_copy(pT, ps_t)
                            for j in range(4):
                                nc.tensor.matmul(
                                    ps_at, pT[:, 128 * j:128 * (j + 1)],
                                    vT_res[4 * n + j],
                                    start=(n == 0 and j == 0),
                                    stop=(n == 7 and j == 3))
                        rowsum = at_s.tile([128, 1], F32, tag="rowsum")
                        nc.vector.reduce_sum(out=rowsum, in_=sums, axis=AX.X)
                        rinv = at_s.tile([128, 1], F32, tag="rinv")
                        nc.vector.reciprocal(out=rinv, in_=rowsum)
                        attn = at_s.tile([128, C], F16, tag="attn")
                        nc.vector.tensor_scalar_mul(out=attn, in0=ps_at,
                                                    scalar1=rinv)
                        # transpose attn [q, c] -> attn_h [c, q]; add bv
                        ps_t2 = pp_t.tile([128, 512], F16, tag="ps_t")
                        for i in range(NT):
                            nc.tensor.transpose(
                                ps_t2[:, 128 * i:128 * (i + 1)],
                                attn[:, 128 * i:128 * (i + 1)], id16_sb)
                        for i in range(NT):
                            nc.vector.tensor_scalar_add(
                                out=attnh4[:, i, 128 * qq:128 * (qq + 1)],
                                in0=ps_t2[:, 128 * i:128 * (i + 1)],
                                scalar1=bv[:, i:i + 1])
                    # batched output projection (fp16, N=512) + bias + residual
                    gcols = slice(512 * g, 512 * (g + 1))
                    for m in range(NT):
                        ps_o = pp_o.tile([128, 512], F32, tag="ps_o")
                        for kc in range(NT):
                            nc.tensor.matmul(
                                ps_o, wo_sb[kc][:, 128 * m:128 * (m + 1)],
                                attnh4[:, kc, :], start=(kc == 0), stop=(kc == 3))
                        xres = at_o.tile([128, 512], F32, tag="xres")
                        nc.sync.dma_start(
                            out=xres, in_=x_in[128 * m:128 * (m + 1), gcols])
                        o_sb = at_o.tile([128, 512], F32, tag="o_sb")
                        nc.vector.scalar_tensor_tensor(
                            out=o_sb, in0=ps_o, scalar=bo[:, m:m + 1],
                            in1=xres, op0=OP.add, op1=OP.add)
                        nc.sync.dma_start(
                            out=out_dram[128 * m:128 * (m + 1), gcols], in_=o_sb)

    nc.compile()
    return nc


_NC_CACHE = None
_NC_BIAS_CACHE = None


def _prep_inputs(inputs):
    x = np.asarray(inputs["x"], np.float32)

    def tile4(v):
        return np.asarray(v, np.float32).reshape(4, 128).T

    biases = np.concatenate(
        [tile4(inputs[k]) for k in ("bq", "bk", "bv", "bo")], axis=1)
    gb = np.concatenate(
        [tile4(inputs["gn_gamma"]), tile4(inputs["gn_beta"])], axis=1)
    wq = np.asarray(inputs["wq"], np.float64)
    wk = np.asarray(inputs["wk"], np.float64)
    mT = np.ascontiguousarray((wk.T @ wq).astype(np.float32))
    shared = {
        "mT": mT,
        "wvT": np.ascontiguousarray(np.asarray(inputs["wv"], np.float32).T),
        "woT16": np.ascontiguousarray(
            np.asarray(inputs["wo"], np.float32).T.astype(np.float16)),
        "biases": np.ascontiguousarray(biases),
        "gammabeta": np.ascontiguousarray(gb),
    }
    if np.any(np.asarray(inputs["bq"], np.float32)) or \
       np.any(np.asarray(inputs["bk"], np.float32)):
        u = wk.T @ np.asarray(inputs["bq"], np.float64)
        shared["uT"] = np.ascontiguousarray(
            u.astype(np.float32).reshape(C, 1))
    return [dict(shared, x=np.ascontiguousarray(x[i].reshape(C, HW)))
            for i in range(x.shape[0])]


def kernel(**inputs):
    global _NC_CACHE, _NC_BIAS_CACHE
    x = np.asarray(inputs["x"], np.float32)
    b, c, h, w = x.shape
    in_maps = _prep_inputs(inputs)
    if "uT" in in_maps[0]:
        if _NC_BIAS_CACHE is None:
            _NC_BIAS_CACHE = build(with_qk_bias=True)
        nc = _NC_BIAS_CACHE
    else:
        if _NC_CACHE is None:
            _NC_CACHE = build(with_qk_bias=False)
        nc = _NC_CACHE
    res = run_bass_kernel_spmd(nc, in_maps, list(range(b)))
    out = np.stack([res.results[i]["out"].reshape(c, h, w) for i in range(b)])
    return out.astype(np.float32)


if __name__ == "__main__":
    import time
    t0 = time.time()
    build()
    print(f"build ok in {time.time()-t0:.1f}s")


# revision 12
# speedup vs baseline: 1.9252x; 1.0298x over previous
"""NonLocalBlock (GroupNorm + single-head 4096x4096 attention + residual)
Trainium2 Bass kernel, data-parallel over batch: 1 image per NeuronCore x8.

Per image (x: [512, 4096] channels-major):
  pass0: GroupNorm stats (bn_stats per channel, group-combine via tiny matmuls)
  passA: per hw-chunk of 512: normalize -> h (fp32r), h16 = fp16(h) resident,
         kk = (Wq^T Wk) @ h via fp32r 1-pass (M precomputed on host in fp64),
         kk16 resident, vT (fp32r 1-pass) -> fp16 resident.
  attention: logits = h16^T @ kk16 (1-pass fp16 matmuls straight into PSUM,
         all 8 chunks of a q-tile live in PSUM banks), row max on DVE from
         PSUM, ACT exp reads PSUM -> fp16 probs (+exact fp32 row sums),
         fp16 PE-transpose probs, fp16 attn@v, fp16 transpose attn_h (+bv),
         fp16 output projection, +bo +residual fused on DVE, store.
  q is never materialized: softmax(q^T k) == softmax(h^T (Wq^T Wk) h); the
  q-row bias term q^T bk is softmax-invariant; bq column term handled in the
  with_qk_bias build variant (bq/bk are zero in practice).
"""
import sys

sys.path.insert(0, '/opt/trn_rl_repo')
import numpy as np
import concourse.bass as bass
import concourse.bacc as bacc
import concourse.mybir as mybir
import concourse.tile as tile
from concourse.bass_utils import run_bass_kernel_spmd

F32 = mybir.dt.float32
F32R = mybir.dt.float32r
F16 = mybir.dt.float16
AF = mybir.ActivationFunctionType
AX = mybir.AxisListType
OP = mybir.AluOpType

C = 512
HW = 4096
NT = 4            # channel tiles of 128
NCH = 8           # hw chunks of 512
NQT = 32          # q tiles of 128
GSIZE = 16        # channels per group
EPS = 1e-5
SCALE = float(np.float32(512.0) ** 0.5)


def build(with_qk_bias=False):
    nc = bacc.Bacc('TRN2', target_bir_lowering=False, debug=False)

    x_in = nc.declare_dram_parameter("x", [C, HW], F32, isOutput=False)
    mT_in = nc.declare_dram_parameter("mT", [C, C], F32, isOutput=False)
    wvT_in = nc.declare_dram_parameter("wvT", [C, C], F32, isOutput=False)
    wo16_in = nc.declare_dram_parameter("woT16", [C, C], F16, isOutput=False)
    bias_in = nc.declare_dram_parameter("biases", [128, 16], F32,
                                        isOutput=False)  # bq|bk|bv|bo as [128,4]
    gb_in = nc.declare_dram_parameter("gammabeta", [128, 8], F32,
                                      isOutput=False)  # gamma|beta as [128,4]
    if with_qk_bias:
        u_in = nc.declare_dram_parameter("uT", [C, 1], F32, isOutput=False)
    out_dram = nc.declare_dram_parameter("out", [C, HW], F32, isOutput=True)

    a16 = np.zeros((128, 8), np.float32)
    for p in range(128):
        a16[p, p // GSIZE] = 1.0 / GSIZE
    b8 = np.zeros((8, 128), np.float32)
    for p in range(128):
        b8[p // GSIZE, p] = 1.0
    a16_d = nc.inline_tensor(a16, "a16")
    b8_d = nc.inline_tensor(b8, "b8")
    ident_d = nc.inline_tensor(np.eye(128, dtype=np.float32), "ident128")
    id16_d = nc.inline_tensor(np.eye(128, dtype=np.float16), "ident16")

    with tile.TileContext(nc) as tc:
        with (
            tc.tile_pool(name="res", bufs=1) as res,
            tc.tile_pool(name="pp_log", bufs=3, space="PSUM") as pp_log,
            tc.tile_pool(name="pp_t", bufs=2, space="PSUM") as pp_t,
            tc.tile_pool(name="pp_at", bufs=1, space="PSUM") as pp_at,
            tc.tile_pool(name="pp_o", bufs=2, space="PSUM") as pp_o,
        ):
            # ---------- residents ----------
            h16_res = [res.tile([128, HW], F16, tag=f"h16_{t}", name=f"h16_{t}")
                       for t in range(NT)]
            kk16_res = [res.tile([128, HW], F16, tag=f"kk{t}", name=f"kk{t}")
                        for t in range(NT)]
            vT_res = [res.tile([128, C], F16, tag=f"vT{m}", name=f"vT{m}")
                      for m in range(NQT)]
            mT_sb = [res.tile([128, C], F32R, tag=f"mT{t}", name=f"mT{t}")
                     for t in range(NT)]
            wv_sb = [res.tile([128, C], F32R, tag=f"wv{t}", name=f"wv{t}")
                     for t in range(NT)]
            wo_sb = [res.tile([128, C], F16, tag=f"wo{t}", name=f"wo{t}")
                     for t in range(NT)]
            for t in range(NT):
                sl = slice(128 * t, 128 * (t + 1))
                nc.gpsimd.dma_start(out=mT_sb[t], in_=mT_in[sl, :])
                nc.gpsimd.dma_start(out=wv_sb[t], in_=wvT_in[sl, :])
                nc.sync.dma_start(out=wo_sb[t], in_=wo16_in[sl, :])
            biases = res.tile([128, 16], F32, tag="biases")
            nc.sync.dma_start(out=biases, in_=bias_in[:])
            bv = biases[:, 8:12]
            bo = biases[:, 12:16]
            gmbt = res.tile([128, 8], F32, tag="gmbt")
            nc.sync.dma_start(out=gmbt, in_=gb_in[:])
            gam = gmbt[:, 0:4]
            bet = gmbt[:, 4:8]
            a16_sb = res.tile([128, 8], F32, tag="a16")
            nc.sync.dma_start(out=a16_sb, in_=a16_d[:])
            b8_sb = res.tile([8, 128], F32, tag="b8")
            nc.sync.dma_start(out=b8_sb, in_=b8_d[:])
            id_sb = res.tile([128, 128], F32, tag="ident")
            nc.sync.dma_start(out=id_sb, in_=ident_d[:])
            id16_sb = res.tile([128, 128], F16, tag="ident16")
            nc.sync.dma_start(out=id16_sb, in_=id16_d[:])
            if with_qk_bias:
                u_sb = res.tile([128, NT], F32R, tag="u_sb")
                nc.gpsimd.dma_start(
                    out=u_sb, in_=u_in[:].rearrange("(t p) o -> p (t o)", p=128))
                ones_col = res.tile([1, 128], F32R, tag="ones_col")
                nc.vector.memset(ones_col, 1.0)
            eps8 = res.tile([8, 1], F32, tag="eps8")
            nc.vector.memset(eps8, EPS)
            scale_sb = res.tile([128, NT], F32, tag="scale")
            shift_sb = res.tile([128, NT], F32, tag="shift")

            # PE warmup: dummy transposes interleaved through pass0 so HAM
            # stays unthrottled (1.2->2.4GHz) until passA matmuls start.
            wps = pp_log.tile([128, 128], F32, tag="ps_l", name="wps")
            for _ in range(24):
                nc.tensor.transpose(wps, id_sb, id_sb)

            # ---------- pass 0: GroupNorm statistics ----------
            with tc.tile_pool(name="p0", bufs=4) as p0, \
                 tc.tile_pool(name="p0s", bufs=1) as p0s:
                st6 = p0s.tile([128, NT, NCH, 6], F32, tag="st6")
                for n in range(NCH):
                    for t in range(NT):
                        xc = p0.tile([128, 512], F32, tag="x0")
                        nc.sync.dma_start(
                            out=xc,
                            in_=x_in[128 * t:128 * (t + 1), 512 * n:512 * (n + 1)])
                        nc.vector.bn_stats(out=st6[:, t, n, :], in_=xc)
                        # keep-warm: depends on xc's DMA, so it lands mid-pass0
                        for _ in range(4):
                            nc.tensor.transpose(wps, xc[:, 0:128], id_sb)
                mv = p0s.tile([128, NT, 2], F32, tag="mv")
                for t in range(NT):
                    nc.vector.bn_aggr(out=mv[:, t, :], in_=st6[:, t, :, :])
                # stats_in: cols 0-3 mean_t, cols 4-7 E[x^2]_t
                stats_in = p0s.tile([128, 8], F32, tag="stats_in")
                for t in range(NT):
                    nc.vector.tensor_copy(stats_in[:, t:t + 1], mv[:, t, 0:1])
                    nc.vector.tensor_mul(stats_in[:, 4 + t:5 + t],
                                         mv[:, t, 0:1], mv[:, t, 0:1])
                    nc.vector.tensor_add(stats_in[:, 4 + t:5 + t],
                                         stats_in[:, 4 + t:5 + t], mv[:, t, 1:2])
                ps_g = pp_o.tile([8, 8], F32, tag="ps_o")
                nc.tensor.matmul(ps_g, a16_sb, stats_in, start=True, stop=True)
                g_sb = p0s.tile([8, 8], F32, tag="g_sb")
                nc.vector.tensor_copy(g_sb, ps_g)
                # group var = E[x^2]_g - mean_g^2 ; rstd = exp(-0.5*ln(var+eps))
                var_g = p0s.tile([8, 4], F32, tag="var_g")
                nc.vector.tensor_mul(var_g, g_sb[:, 0:4], g_sb[:, 0:4])
                nc.vector.tensor_tensor(out=var_g, in0=g_sb[:, 4:8], in1=var_g,
                                        op=OP.subtract)
                bc_in = p0s.tile([8, 8], F32, tag="bc_in")
                nc.vector.tensor_copy(bc_in[:, 0:4], g_sb[:, 0:4])
                nc.scalar.activation(out=bc_in[:, 4:8], in_=var_g, func=AF.Ln,
                                     bias=eps8, scale=1.0)
                nc.scalar.activation(out=bc_in[:, 4:8], in_=bc_in[:, 4:8],
                                     func=AF.Exp, bias=0.0, scale=-0.5)
                ps_bc = pp_o.tile([128, 8], F32, tag="ps_o")
                nc.tensor.matmul(ps_bc, b8_sb, bc_in, start=True, stop=True)
                chan = p0s.tile([128, 8], F32, tag="chan")
                nc.vector.tensor_copy(chan, ps_bc)
                # scale = gamma * rstd ; shift = beta - mean*scale
                nc.vector.tensor_mul(scale_sb, gam, chan[:, 4:8])
                tmp = p0s.tile([128, NT], F32, tag="tmp")
                nc.vector.tensor_mul(tmp, chan[:, 0:4], scale_sb)
                nc.vector.tensor_tensor(out=shift_sb, in0=bet, in1=tmp,
                                        op=OP.subtract)

            # ---------- pass A: hidden -> h16, kk16, vT16 (+u row) ----------
            with tc.tile_pool(name="pa_x", bufs=3) as pa_x, \
                 tc.tile_pool(name="pa_hr", bufs=8) as pa_hr:
                if with_qk_bias:
                    r_row = res.tile([1, HW], F32, tag="r_row")
                for n in range(NCH):
                    cols = slice(512 * n, 512 * (n + 1))
                    hid_r = []
                    for t in range(NT):
                        xc = pa_x.tile([128, 512], F32, tag="xA")
                        nc.sync.dma_start(
                            out=xc, in_=x_in[128 * t:128 * (t + 1), cols])
                        hr = pa_hr.tile([128, 512], F32R, tag="hid_r", bufs=8)
                        nc.vector.tensor_scalar(
                            out=hr, in0=xc,
                            scalar1=scale_sb[:, t:t + 1],
                            scalar2=shift_sb[:, t:t + 1],
                            op0=OP.mult, op1=OP.add)
                        hid_r.append(hr)
                        nc.scalar.copy(out=h16_res[t][:, cols], in_=hr)
                    # kk = M @ h (fp32r 1-pass), round to fp16
                    for t in range(NT):
                        ps = pp_o.tile([128, 512], F32, tag="ps_o")
                        for kc in range(NT):
                            nc.tensor.matmul(
                                ps, mT_sb[kc][:, 128 * t:128 * (t + 1)],
                                hid_r[kc], start=(kc == 0), stop=(kc == 3))
                        nc.scalar.copy(out=kk16_res[t][:, cols], in_=ps)
                    # vT (fp32r 1-pass): out[hw_t 128, c 512], round to fp16
                    # (bv folded into attn_h later: softmax weights sum to 1)
                    for t in range(NT):
                        ps = pp_o.tile([128, 512], F32, tag="ps_o")
                        for kc in range(NT):
                            nc.tensor.matmul(
                                ps, hid_r[kc][:, 128 * t:128 * (t + 1)],
                                wv_sb[kc], start=(kc == 0), stop=(kc == 3))
                        nc.vector.tensor_copy(vT_res[4 * n + t], ps)
                    if with_qk_bias:
                        # r = u^T h  [1, 512] chunk (bq column term)
                        ps_r = pp_log.tile([1, 512], F32, tag="ps_l")
                        for kc in range(NT):
                            nc.tensor.matmul(
                                ps_r, u_sb[:, kc:kc + 1], hid_r[kc],
                                start=(kc == 0), stop=(kc == 3))
                        nc.vector.tensor_copy(r_row[:, cols], ps_r)

            # ---------- attention (software-pipelined over q-tiles) ----------
            # stage qt:   logits matmuls -> PSUM, chunk maxes, stage to SBUF
            # stage qt-1: softmax tail: exp -> fp16 probs -> transpose -> attn@v
            # stage qt-2: attn_h transpose + bv add into the group buffer
            # Issuing the tails AFTER the next q-tile's logits keeps the
            # in-order Tensor queue from stalling on the ACT exp latency.
            with tc.tile_pool(name="at_l", bufs=2) as at_l, \
                 tc.tile_pool(name="at_p", bufs=3) as at_p, \
                 tc.tile_pool(name="at_pt", bufs=3) as at_pt, \
                 tc.tile_pool(name="at_s", bufs=2) as at_s, \
                 tc.tile_pool(name="at_h4", bufs=2) as at_h4, \
                 tc.tile_pool(name="at_o", bufs=2) as at_o:
                lgs, maxss, attns, h4s, xress = {}, {}, {}, {}, {}

                def logits_stage(qt):
                    qcols = slice(128 * qt, 128 * (qt + 1))
                    lg = at_l.tile([128, HW], F32, tag="lg")
                    maxs = at_s.tile([128, NCH], F32, tag="maxs")
                    for n in range(NCH):
                        ncols = slice(512 * n, 512 * (n + 1))
                        ps_l = pp_log.tile([128, 512], F32, tag="ps_l")
                        for kc in range(NT):
                            nc.tensor.matmul(
                                ps_l, h16_res[kc][:, qcols],
                                kk16_res[kc][:, ncols],
                                start=(kc == 0), stop=(kc == 3 and
                                                       not with_qk_bias))
                        if with_qk_bias:
                            nc.tensor.matmul(
                                ps_l, ones_col, r_row[:, ncols],
                                start=False, stop=True)
                        nc.vector.reduce_max(out=maxs[:, n:n + 1], in_=ps_l,
                                             axis=AX.X)
                        if n % 2 == 0:
                            nc.scalar.copy(out=lg[:, ncols], in_=ps_l)
                        else:
                            nc.vector.tensor_copy(out=lg[:, ncols], in_=ps_l)
                    lgs[qt], maxss[qt] = lg, maxs

                def softmax_av_stage(qt):
                    lg, maxs = lgs.pop(qt), maxss.pop(qt)
                    negmax = at_s.tile([128, 1], F32, tag="negmax")
                    nc.vector.reduce_max(out=negmax, in_=maxs, axis=AX.X,
                                         negate=True)
                    negmax_s = at_s.tile([128, 1], F32, tag="negmax_s")
                    nc.vector.tensor_scalar_mul(out=negmax_s, in0=negmax,
                                                scalar1=SCALE)
                    sums = at_s.tile([128, NCH], F32, tag="sums")
                    ps_at = pp_at.tile([128, C], F32, tag="ps_at")
                    for n in range(NCH):
                        probs = at_p.tile([128, 512], F16, tag="probs")
                        nc.scalar.activation(
                            out=probs, in_=lg[:, 512 * n:512 * (n + 1)],
                            func=AF.Exp, bias=negmax_s, scale=SCALE,
                            accum_out=sums[:, n:n + 1])
                        ps_t = pp_t.tile([128, 512], F16, tag="ps_t")
                        for j in range(4):
                            nc.tensor.transpose(
                                ps_t[:, 128 * j:128 * (j + 1)],
                                probs[:, 128 * j:128 * (j + 1)], id16_sb)
                        pT = at_pt.tile([128, 512], F16, tag="pT")
                        nc.vector.tensor_copy(pT, ps_t)
                        for j in range(4):
                            nc.tensor.matmul(
                                ps_at, pT[:, 128 * j:128 * (j + 1)],
                                vT_res[4 * n + j],
                                start=(n == 0 and j == 0),
                                stop=(n == 7 and j == 3))
                    rowsum = at_s.tile([128, 1], F32, tag="rowsum")
                    nc.vector.reduce_sum(out=rowsum, in_=sums, axis=AX.X)
                    rinv = at_s.tile([128, 1], F32, tag="rinv")
                    nc.vector.reciprocal(out=rinv, in_=rowsum)
                    attn = at_s.tile([128, C], F16, tag="attn")
                    nc.vector.tensor_scalar_mul(out=attn, in0=ps_at,
                                                scalar1=rinv)
                    attns[qt] = attn

                def attnh_stage(qt):
                    attn = attns.pop(qt)
                    attnh4 = h4s[qt // 4]
                    qq = qt % 4
                    ps_t2 = pp_t.tile([128, 512], F16, tag="ps_t")
                    for i in range(NT):
                        nc.tensor.transpose(
                            ps_t2[:, 128 * i:128 * (i + 1)],
                            attn[:, 128 * i:128 * (i + 1)], id16_sb)
                    for i in range(NT):
                        nc.vector.tensor_scalar_add(
                            out=attnh4[:, i, 128 * qq:128 * (qq + 1)],
                            in0=ps_t2[:, 128 * i:128 * (i + 1)],
                            scalar1=bv[:, i:i + 1])

                def outproj_stage(g):
                    attnh4 = h4s.pop(g)
                    gcols = slice(512 * g, 512 * (g + 1))
                    for m in range(NT):
                        ps_o = pp_o.tile([128, 512], F32, tag="ps_o")
                        for kc in range(NT):
                            nc.tensor.matmul(
                                ps_o, wo_sb[kc][:, 128 * m:128 * (m + 1)],
                                attnh4[:, kc, :], start=(kc == 0), stop=(kc == 3))
                        o_sb = at_o.tile([128, 512], F32, tag="o_sb")
                        nc.vector.scalar_tensor_tensor(
                            out=o_sb, in0=ps_o, scalar=bo[:, m:m + 1],
                            in1=xress[g][m], op0=OP.add, op1=OP.add)
                        nc.sync.dma_start(
                            out=out_dram[128 * m:128 * (m + 1), gcols], in_=o_sb)
                    del xress[g]

                for qt in range(NQT + 2):
                    if qt < NQT:
                        if qt % 4 == 0:
                            g = qt // 4
                            h4s[g] = at_h4.tile([128, NT, 512], F16,
                                                tag="attnh4", name=f"ah4_{g}")
                            xres = [at_o.tile([128, 512], F32, tag="xres",
                                              bufs=8, name=f"xres{g}_{m}")
                                    for m in range(NT)]
                            for m in range(NT):
                                nc.sync.dma_start(
                                    out=xres[m],
                                    in_=x_in[128 * m:128 * (m + 1),
                                             512 * g:512 * (g + 1)])
                            xress[g] = xres
                        logits_stage(qt)
                    if 1 <= qt <= NQT:
                        softmax_av_stage(qt - 1)
                    if qt >= 2:
                        attnh_stage(qt - 2)
                        if (qt - 2) % 4 == 3:
                            outproj_stage((qt - 2) // 4)

    nc.compile()
    return nc


_NC_CACHE = None
_NC_BIAS_CACHE = None


def _prep_inputs(inputs):
    x = np.asarray(inputs["x"], np.float32)

    def tile4(v):
        return np.asarray(v, np.float32).reshape(4, 128).T

    biases = np.concatenate(
        [tile4(inputs[k]) for k in ("bq", "bk", "bv", "bo")], axis=1)
    gb = np.concatenate(
        [tile4(inputs["gn_gamma"]), tile4(inputs["gn_beta"])], axis=1)
    wq = np.asarray(inputs["wq"], np.float64)
    wk = np.asarray(inputs["wk"], np.float64)
    mT = np.ascontiguousarray((wk.T @ wq).astype(np.float32))
    shared = {
        "mT": mT,
        "wvT": np.ascontiguousarray(np.asarray(inputs["wv"], np.float32).T),
        "woT16": np.ascontiguousarray(
            np.asarray(inputs["wo"], np.float32).T.astype(np.float16)),
        "biases": np.ascontiguousarray(biases),
        "gammabeta": np.ascontiguousarray(gb),
    }
    if np.any(np.asarray(inputs["bq"], np.float32)) or \
       np.any(np.asarray(inputs["bk"], np.float32)):
        u = wk.T @ np.asarray(inputs["bq"], np.float64)
        shared["uT"] = np.ascontiguousarray(
            u.astype(np.float32).reshape(C, 1))
    return [dict(shared, x=np.ascontiguousarray(x[i].reshape(C, HW)))
            for i in range(x.shape[0])]


def kernel(**inputs):
    global _NC_CACHE, _NC_BIAS_CACHE
    x = np.asarray(inputs["x"], np.float32)
    b, c, h, w = x.shape
    in_maps = _prep_inputs(inputs)
    if "uT" in in_maps[0]:
        if _NC_BIAS_CACHE is None:
            _NC_BIAS_CACHE = build(with_qk_bias=True)
        nc = _NC_BIAS_CACHE
    else:
        if _NC_CACHE is None:
            _NC_CACHE = build(with_qk_bias=False)
        nc = _NC_CACHE
    res = run_bass_kernel_spmd(nc, in_maps, list(range(b)))
    out = np.stack([res.results[i]["out"].reshape(c, h, w) for i in range(b)])
    return out.astype(np.float32)


if __name__ == "__main__":
    import time
    t0 = time.time()
    build()
    print(f"build ok in {time.time()-t0:.1f}s")


# revision 17
# speedup vs baseline: 1.9464x; 1.0110x over previous
"""NonLocalBlock (GroupNorm + single-head 4096x4096 attention + residual)
Trainium2 Bass kernel, data-parallel over batch: 1 image per NeuronCore x8.

Per image (x: [512, 4096] channels-major):
  pass0: GroupNorm stats (bn_stats per channel, group-combine via tiny matmuls)
  passA: per hw-chunk of 512: normalize -> h (fp32r), h16 = fp16(h) resident,
         kk = (Wq^T Wk) @ h via fp32r 1-pass (M precomputed on host in fp64),
         kk16 resident, vT (fp32r 1-pass) -> fp16 resident.
  attention: logits = h16^T @ kk16 (1-pass fp16 matmuls straight into PSUM,
         all 8 chunks of a q-tile live in PSUM banks), row max on DVE from
         PSUM, ACT exp reads PSUM -> fp16 probs (+exact fp32 row sums),
         fp16 PE-transpose probs, fp16 attn@v, fp16 transpose attn_h (+bv),
         fp16 output projection, +bo +residual fused on DVE, store.
  q is never materialized: softmax(q^T k) == softmax(h^T (Wq^T Wk) h); the
  q-row bias term q^T bk is softmax-invariant; bq column term handled in the
  with_qk_bias build variant (bq/bk are zero in practice).
"""
import sys

sys.path.insert(0, '/opt/trn_rl_repo')
import numpy as np
import concourse.bass as bass
import concourse.bacc as bacc
import concourse.mybir as mybir
import concourse.tile as tile
from concourse.bass_utils import run_bass_kernel_spmd

F32 = mybir.dt.float32
F32R = mybir.dt.float32r
F16 = mybir.dt.float16
AF = mybir.ActivationFunctionType
AX = mybir.AxisListType
OP = mybir.AluOpType

C = 512
HW = 4096
NT = 4            # channel tiles of 128
NCH = 8           # hw chunks of 512
NQT = 32          # q tiles of 128
GSIZE = 16        # channels per group
EPS = 1e-5
SCALE = float(np.float32(512.0) ** 0.5)


def build(with_qk_bias=False):
    nc = bacc.Bacc('TRN2', target_bir_lowering=False, debug=False)

    x_in = nc.declare_dram_parameter("x", [C, HW], F32, isOutput=False)
    mT_in = nc.declare_dram_parameter("mT", [C, C], F32, isOutput=False)
    wvT_in = nc.declare_dram_parameter("wvT", [C, C], F32, isOutput=False)
    wo16_in = nc.declare_dram_parameter("woT16", [C, C], F16, isOutput=False)
    bias_in = nc.declare_dram_parameter("biases", [128, 16], F32,
                                        isOutput=False)  # bq|bk|bv|bo as [128,4]
    gb_in = nc.declare_dram_parameter("gammabeta", [128, 8], F32,
                                      isOutput=False)  # gamma|beta as [128,4]
    if with_qk_bias:
        u_in = nc.declare_dram_parameter("uT", [C, 1], F32, isOutput=False)
    out_dram = nc.declare_dram_parameter("out", [C, HW], F32, isOutput=True)

    a16 = np.zeros((128, 8), np.float32)
    for p in range(128):
        a16[p, p // GSIZE] = 1.0 / GSIZE
    b8 = np.zeros((8, 128), np.float32)
    for p in range(128):
        b8[p // GSIZE, p] = 1.0
    a16_d = nc.inline_tensor(a16, "a16")
    b8_d = nc.inline_tensor(b8, "b8")
    ident_d = nc.inline_tensor(np.eye(128, dtype=np.float32), "ident128")
    id16_d = nc.inline_tensor(np.eye(128, dtype=np.float16), "ident16")

    with tile.TileContext(nc) as tc:
        with (
            tc.tile_pool(name="res", bufs=1) as res,
            tc.tile_pool(name="pp_log", bufs=3, space="PSUM") as pp_log,
            tc.tile_pool(name="pp_t", bufs=2, space="PSUM") as pp_t,
            tc.tile_pool(name="pp_at", bufs=1, space="PSUM") as pp_at,
            tc.tile_pool(name="pp_o", bufs=2, space="PSUM") as pp_o,
        ):
            # ---------- residents ----------
            h16_res = [res.tile([128, HW], F16, tag=f"h16_{t}", name=f"h16_{t}")
                       for t in range(NT)]
            kk16_res = [res.tile([128, HW], F16, tag=f"kk{t}", name=f"kk{t}")
                        for t in range(NT)]
            vT_res = [res.tile([128, C], F16, tag=f"vT{m}", name=f"vT{m}")
                      for m in range(NQT)]
            mT_sb = [res.tile([128, C], F32R, tag=f"mT{t}", name=f"mT{t}")
                     for t in range(NT)]
            wv_sb = [res.tile([128, C], F32R, tag=f"wv{t}", name=f"wv{t}")
                     for t in range(NT)]
            wo_sb = [res.tile([128, C], F16, tag=f"wo{t}", name=f"wo{t}")
                     for t in range(NT)]
            id_sb = res.tile([128, 128], F32, tag="ident")
            nc.sync.dma_start(out=id_sb, in_=ident_d[:])
            id16_sb = res.tile([128, 128], F16, tag="ident16")
            nc.sync.dma_start(out=id16_sb, in_=id16_d[:])
            for t in range(NT):
                sl = slice(128 * t, 128 * (t + 1))
                nc.gpsimd.dma_start(out=mT_sb[t], in_=mT_in[sl, :])
                nc.gpsimd.dma_start(out=wv_sb[t], in_=wvT_in[sl, :])
                nc.sync.dma_start(out=wo_sb[t], in_=wo16_in[sl, :])
            biases = res.tile([128, 16], F32, tag="biases")
            nc.sync.dma_start(out=biases, in_=bias_in[:])
            bv = biases[:, 8:12]
            bo = biases[:, 12:16]
            gmbt = res.tile([128, 8], F32, tag="gmbt")
            nc.sync.dma_start(out=gmbt, in_=gb_in[:])
            gam = gmbt[:, 0:4]
            bet = gmbt[:, 4:8]
            a16_sb = res.tile([128, 8], F32, tag="a16")
            nc.sync.dma_start(out=a16_sb, in_=a16_d[:])
            b8_sb = res.tile([8, 128], F32, tag="b8")
            nc.sync.dma_start(out=b8_sb, in_=b8_d[:])
            if with_qk_bias:
                u_sb = res.tile([128, NT], F32R, tag="u_sb")
                nc.gpsimd.dma_start(
                    out=u_sb, in_=u_in[:].rearrange("(t p) o -> p (t o)", p=128))
                ones_col = res.tile([1, 128], F32R, tag="ones_col")
                nc.vector.memset(ones_col, 1.0)
            eps8 = res.tile([8, 1], F32, tag="eps8")
            nc.vector.memset(eps8, EPS)
            scale_sb = res.tile([128, NT], F32, tag="scale")
            shift_sb = res.tile([128, NT], F32, tag="shift")

            # PE warmup: dummy transposes interleaved through pass0 so HAM
            # stays unthrottled (1.2->2.4GHz) until passA matmuls start.
            wps = pp_log.tile([128, 128], F32, tag="ps_l", name="wps")
            for _ in range(24):
                nc.tensor.transpose(wps, id_sb, id_sb)

            # ---------- pass 0: GroupNorm statistics ----------
            with tc.tile_pool(name="p0", bufs=4) as p0, \
                 tc.tile_pool(name="p0s", bufs=1) as p0s:
                st6 = p0s.tile([128, NT, NCH, 6], F32, tag="st6")
                for n in range(4):
                    for t in range(NT):
                        xc = p0.tile([128, 1024], F32, tag="x0")
                        eng = nc.sync if (t % 2 == 0) else nc.gpsimd
                        eng.dma_start(
                            out=xc,
                            in_=x_in[128 * t:128 * (t + 1),
                                     1024 * n:1024 * (n + 1)])
                        nc.vector.bn_stats(out=st6[:, t, 2 * n, :],
                                           in_=xc[:, 0:512])
                        nc.vector.bn_stats(out=st6[:, t, 2 * n + 1, :],
                                           in_=xc[:, 512:1024])
                        # keep-warm: depends on xc's DMA, so it lands mid-pass0
                        for _ in range(8):
                            nc.tensor.transpose(wps, xc[:, 0:128], id_sb)
                mv = p0s.tile([128, NT, 2], F32, tag="mv")
                for t in range(NT):
                    nc.vector.bn_aggr(out=mv[:, t, :], in_=st6[:, t, :, :])
                # stats_in: cols 0-3 mean_t, cols 4-7 E[x^2]_t
                stats_in = p0s.tile([128, 8], F32, tag="stats_in")
                for t in range(NT):
                    nc.vector.tensor_copy(stats_in[:, t:t + 1], mv[:, t, 0:1])
                    nc.vector.tensor_mul(stats_in[:, 4 + t:5 + t],
                                         mv[:, t, 0:1], mv[:, t, 0:1])
                    nc.vector.tensor_add(stats_in[:, 4 + t:5 + t],
                                         stats_in[:, 4 + t:5 + t], mv[:, t, 1:2])
                ps_g = pp_o.tile([8, 8], F32, tag="ps_o")
                nc.tensor.matmul(ps_g, a16_sb, stats_in, start=True, stop=True)
                g_sb = p0s.tile([8, 8], F32, tag="g_sb")
                nc.vector.tensor_copy(g_sb, ps_g)
                # group var = E[x^2]_g - mean_g^2 ; rstd = exp(-0.5*ln(var+eps))
                var_g = p0s.tile([8, 4], F32, tag="var_g")
                nc.vector.tensor_mul(var_g, g_sb[:, 0:4], g_sb[:, 0:4])
                nc.vector.tensor_tensor(out=var_g, in0=g_sb[:, 4:8], in1=var_g,
                                        op=OP.subtract)
                bc_in = p0s.tile([8, 8], F32, tag="bc_in")
                nc.vector.tensor_copy(bc_in[:, 0:4], g_sb[:, 0:4])
                nc.scalar.activation(out=bc_in[:, 4:8], in_=var_g, func=AF.Ln,
                                     bias=eps8, scale=1.0)
                nc.scalar.activation(out=bc_in[:, 4:8], in_=bc_in[:, 4:8],
                                     func=AF.Exp, bias=0.0, scale=-0.5)
                ps_bc = pp_o.tile([128, 8], F32, tag="ps_o")
                nc.tensor.matmul(ps_bc, b8_sb, bc_in, start=True, stop=True)
                chan = p0s.tile([128, 8], F32, tag="chan")
                nc.vector.tensor_copy(chan, ps_bc)
                # scale = gamma * rstd ; shift = beta - mean*scale
                nc.vector.tensor_mul(scale_sb, gam, chan[:, 4:8])
                tmp = p0s.tile([128, NT], F32, tag="tmp")
                nc.vector.tensor_mul(tmp, chan[:, 0:4], scale_sb)
                nc.vector.tensor_tensor(out=shift_sb, in0=bet, in1=tmp,
                                        op=OP.subtract)

            # ---------- pass A: hidden -> h16, kk16, vT16 (+u row) ----------
            with tc.tile_pool(name="pa_x", bufs=6) as pa_x, \
                 tc.tile_pool(name="pa_hr", bufs=8) as pa_hr:
                if with_qk_bias:
                    r_row = res.tile([1, HW], F32, tag="r_row")
                for n in range(NCH):
                    cols = slice(512 * n, 512 * (n + 1))
                    hid_r = []
                    for t in range(NT):
                        xc = pa_x.tile([128, 512], F32, tag="xA")
                        eng = nc.sync if (t % 2 == 0) else nc.gpsimd
                        eng.dma_start(
                            out=xc, in_=x_in[128 * t:128 * (t + 1), cols])
                        hr = pa_hr.tile([128, 512], F32R, tag="hid_r", bufs=8)
                        nc.vector.tensor_scalar(
                            out=hr, in0=xc,
                            scalar1=scale_sb[:, t:t + 1],
                            scalar2=shift_sb[:, t:t + 1],
                            op0=OP.mult, op1=OP.add)
                        hid_r.append(hr)
                        nc.scalar.copy(out=h16_res[t][:, cols], in_=hr)
                    # kk = M @ h (fp32r 1-pass), round to fp16
                    for t in range(NT):
                        ps = pp_o.tile([128, 512], F32, tag="ps_o")
                        for kc in range(NT):
                            nc.tensor.matmul(
                                ps, mT_sb[kc][:, 128 * t:128 * (t + 1)],
                                hid_r[kc], start=(kc == 0), stop=(kc == 3))
                        nc.scalar.copy(out=kk16_res[t][:, cols], in_=ps)
                    # vT (fp32r 1-pass): out[hw_t 128, c 512], round to fp16
                    # (bv folded into attn_h later: softmax weights sum to 1)
                    for t in range(NT):
                        ps = pp_o.tile([128, 512], F32, tag="ps_o")
                        for kc in range(NT):
                            nc.tensor.matmul(
                                ps, hid_r[kc][:, 128 * t:128 * (t + 1)],
                                wv_sb[kc], start=(kc == 0), stop=(kc == 3))
                        nc.vector.tensor_copy(vT_res[4 * n + t], ps)
                    if with_qk_bias:
                        # r = u^T h  [1, 512] chunk (bq column term)
                        ps_r = pp_log.tile([1, 512], F32, tag="ps_l")
                        for kc in range(NT):
                            nc.tensor.matmul(
                                ps_r, u_sb[:, kc:kc + 1], hid_r[kc],
                                start=(kc == 0), stop=(kc == 3))
                        nc.vector.tensor_copy(r_row[:, cols], ps_r)

            # ---------- attention (software-pipelined over q-tiles) ----------
            # stage qt:   logits matmuls -> PSUM, chunk maxes, stage to SBUF
            # stage qt-1: softmax tail: exp -> fp16 probs -> transpose -> attn@v
            # stage qt-2: attn_h transpose + bv add into the group buffer
            # Issuing the tails AFTER the next q-tile's logits keeps the
            # in-order Tensor queue from stalling on the ACT exp latency.
            with tc.tile_pool(name="at_l", bufs=2) as at_l, \
                 tc.tile_pool(name="at_p", bufs=3) as at_p, \
                 tc.tile_pool(name="at_pt", bufs=3) as at_pt, \
                 tc.tile_pool(name="at_s", bufs=2) as at_s, \
                 tc.tile_pool(name="at_h4", bufs=2) as at_h4, \
                 tc.tile_pool(name="at_o", bufs=2) as at_o:
                lgs, maxss, attns, h4s, xress = {}, {}, {}, {}, {}

                def logits_stage(qt):
                    qcols = slice(128 * qt, 128 * (qt + 1))
                    lg = at_l.tile([128, HW], F32, tag="lg")
                    maxs = at_s.tile([128, NCH], F32, tag="maxs")
                    for n in range(NCH):
                        ncols = slice(512 * n, 512 * (n + 1))
                        ps_l = pp_log.tile([128, 512], F32, tag="ps_l")
                        for kc in range(NT):
                            nc.tensor.matmul(
                                ps_l, h16_res[kc][:, qcols],
                                kk16_res[kc][:, ncols],
                                start=(kc == 0), stop=(kc == 3 and
                                                       not with_qk_bias))
                        if with_qk_bias:
                            nc.tensor.matmul(
                                ps_l, ones_col, r_row[:, ncols],
                                start=False, stop=True)
                        nc.vector.reduce_max(out=maxs[:, n:n + 1], in_=ps_l,
                                             axis=AX.X)
                        if n % 2 == 0:
                            nc.scalar.copy(out=lg[:, ncols], in_=ps_l)
                        else:
                            nc.vector.tensor_copy(out=lg[:, ncols], in_=ps_l)
                    lgs[qt], maxss[qt] = lg, maxs

                def softmax_av_stage(qt):
                    lg, maxs = lgs.pop(qt), maxss.pop(qt)
                    negmax = at_s.tile([128, 1], F32, tag="negmax")
                    nc.vector.reduce_max(out=negmax, in_=maxs, axis=AX.X,
                                         negate=True)
                    negmax_s = at_s.tile([128, 1], F32, tag="negmax_s")
                    nc.vector.tensor_scalar_mul(out=negmax_s, in0=negmax,
                                                scalar1=SCALE)
                    sums = at_s.tile([128, NCH], F32, tag="sums")
                    ps_at = pp_at.tile([128, C], F32, tag="ps_at")
                    for n in range(NCH):
                        probs = at_p.tile([128, 512], F16, tag="probs")
                        nc.scalar.activation(
                            out=probs, in_=lg[:, 512 * n:512 * (n + 1)],
                            func=AF.Exp, bias=negmax_s, scale=SCALE,
                            accum_out=sums[:, n:n + 1])
                        ps_t = pp_t.tile([128, 512], F16, tag="ps_t")
                        for j in range(4):
                            nc.tensor.transpose(
                                ps_t[:, 128 * j:128 * (j + 1)],
                                probs[:, 128 * j:128 * (j + 1)], id16_sb)
                        pT = at_pt.tile([128, 512], F16, tag="pT")
                        nc.vector.tensor_copy(pT, ps_t)
                        for j in range(4):
                            nc.tensor.matmul(
                                ps_at, pT[:, 128 * j:128 * (j + 1)],
                                vT_res[4 * n + j],
                                start=(n == 0 and j == 0),
                                stop=(n == 7 and j == 3))
                    rowsum = at_s.tile([128, 1], F32, tag="rowsum")
                    nc.vector.reduce_sum(out=rowsum, in_=sums, axis=AX.X)
                    rinv = at_s.tile([128, 1], F32, tag="rinv")
                    nc.vector.reciprocal(out=rinv, in_=rowsum)
                    attn = at_s.tile([128, C], F16, tag="attn")
                    nc.vector.tensor_scalar_mul(out=attn, in0=ps_at,
                                                scalar1=rinv)
                    attns[qt] = attn

                def attnh_stage(qt):
                    attn = attns.pop(qt)
                    attnh4 = h4s[qt // 4]
                    qq = qt % 4
                    ps_t2 = pp_t.tile([128, 512], F16, tag="ps_t")
                    for i in range(NT):
                        nc.tensor.transpose(
                            ps_t2[:, 128 * i:128 * (i + 1)],
                            attn[:, 128 * i:128 * (i + 1)], id16_sb)
                    for i in range(NT):
                        nc.vector.tensor_scalar_add(
                            out=attnh4[:, i, 128 * qq:128 * (qq + 1)],
                            in0=ps_t2[:, 128 * i:128 * (i + 1)],
                            scalar1=bv[:, i:i + 1])

                def outproj_stage(g):
                    attnh4 = h4s.pop(g)
                    gcols = slice(512 * g, 512 * (g + 1))
                    for m in range(NT):
                        ps_o = pp_o.tile([128, 512], F32, tag="ps_o")
                        for kc in range(NT):
                            nc.tensor.matmul(
                                ps_o, wo_sb[kc][:, 128 * m:128 * (m + 1)],
                                attnh4[:, kc, :], start=(kc == 0), stop=(kc == 3))
                        o_sb = at_o.tile([128, 512], F32, tag="o_sb")
                        nc.vector.scalar_tensor_tensor(
                            out=o_sb, in0=ps_o, scalar=bo[:, m:m + 1],
                            in1=xress[g][m], op0=OP.add, op1=OP.add)
                        nc.sync.dma_start(
                            out=out_dram[128 * m:128 * (m + 1), gcols], in_=o_sb)
                    del xress[g]

                for qt in range(NQT + 2):
                    if qt < NQT:
                        if qt % 4 == 0:
                            g = qt // 4
                            h4s[g] = at_h4.tile([128, NT, 512], F16,
                                                tag="attnh4", name=f"ah4_{g}")
                            xres = [at_o.tile([128, 512], F32, tag="xres",
                                              bufs=8, name=f"xres{g}_{m}")
                                    for m in range(NT)]
                            for m in range(NT):
                                nc.sync.dma_start(
                                    out=xres[m],
                                    in_=x_in[128 * m:128 * (m + 1),
                                             512 * g:512 * (g + 1)])
                            xress[g] = xres
                        logits_stage(qt)
                    if 1 <= qt <= NQT:
                        softmax_av_stage(qt - 1)
                    if qt >= 2:
                        attnh_stage(qt - 2)
                        if (qt - 2) % 4 == 3:
                            outproj_stage((qt - 2) // 4)

    nc.compile()
    return nc


_NC_CACHE = None
_NC_BIAS_CACHE = None


def _prep_inputs(inputs):
    x = np.asarray(inputs["x"], np.float32)

    def tile4(v):
        return np.asarray(v, np.float32).reshape(4, 128).T

    biases = np.concatenate(
        [tile4(inputs[k]) for k in ("bq", "bk", "bv", "bo")], axis=1)
    gb = np.concatenate(
        [tile4(inputs["gn_gamma"]), tile4(inputs["gn_beta"])], axis=1)
    wq = np.asarray(inputs["wq"], np.float64)
    wk = np.asarray(inputs["wk"], np.float64)
    mT = np.ascontiguousarray((wk.T @ wq).astype(np.float32))
    shared = {
        "mT": mT,
        "wvT": np.ascontiguousarray(np.asarray(inputs["wv"], np.float32).T),
        "woT16": np.ascontiguousarray(
            np.asarray(inputs["wo"], np.float32).T.astype(np.float16)),
        "biases": np.ascontiguousarray(biases),
        "gammabeta": np.ascontiguousarray(gb),
    }
    if np.any(np.asarray(inputs["bq"], np.float32)) or \
       np.any(np.asarray(inputs["bk"], np.float32)):
        u = wk.T @ np.asarray(inputs["bq"], np.float64)
        shared["uT"] = np.ascontiguousarray(
            u.astype(np.float32).reshape(C, 1))
    return [dict(shared, x=np.ascontiguousarray(x[i].reshape(C, HW)))
            for i in range(x.shape[0])]


def kernel(**inputs):
    global _NC_CACHE, _NC_BIAS_CACHE
    x = np.asarray(inputs["x"], np.float32)
    b, c, h, w = x.shape
    in_maps = _prep_inputs(inputs)
    if "uT" in in_maps[0]:
        if _NC_BIAS_CACHE is None:
            _NC_BIAS_CACHE = build(with_qk_bias=True)
        nc = _NC_BIAS_CACHE
    else:
        if _NC_CACHE is None:
            _NC_CACHE = build(with_qk_bias=False)
        nc = _NC_CACHE
    res = run_bass_kernel_spmd(nc, in_maps, list(range(b)))
    out = np.stack([res.results[i]["out"].reshape(c, h, w) for i in range(b)])
    return out.astype(np.float32)


if __name__ == "__main__":
    import time
    t0 = time.time()
    build()
    print(f"build ok in {time.time()-t0:.1f}s")
